# revision 2
# baseline (speedup 1.0000x reference)
"""Trainium2 Bass kernel for nn_CILRSVAEModel (CILRS-style MoE branch routing).

Model (per sample):
  s      = relu(speed @ si_W1 + si_b1) @ si_W2 + si_b2            # [SL]
  emb    = concat(embedding, s)                                   # [D=640]
  branch = command-selected MLP: sigmoid(relu(relu(emb@W1+b1)@W2+b2)@W3+b3)
  speed_pred = relu(relu(emb@so_W1+so_b1)@so_W2+so_b2)@so_W3+so_b3

Strategy: pure data parallel over batch, plus host-side MoE routing — samples
are bucketed by `command` into single-command tiles of 512, so the device
computes only the selected branch per sample (6x fewer branch FLOPs than the
dense reference). Each tile's branch weights are gathered host-side into a
per-tile weight array, keeping the device program fully static. The embedding
shard is pre-transposed on the host to [E, cap] so activations live in
[feature, batch] layout end-to-end (no on-device transposes); matmuls run as
lhsT=[K,M] weights x rhs=[K,N] activations with N=512 batch columns in fp32r.
"""

import os

# The device program runs through jax's axon/PJRT backend; make sure a
# harness-set JAX_PLATFORMS=cpu doesn't hide the neuron devices.
if os.environ.get("JAX_PLATFORMS", None) in ("cpu",):
    os.environ.pop("JAX_PLATFORMS")

import sys

import numpy as np

for _p in ("/opt/trn_rl_repo",):
    if _p not in sys.path and os.path.isdir(_p):
        sys.path.insert(0, _p)

B = 65536
E = 512
SL = 128
H = 256
NB = 6
D = E + SL
NCORES = 8
NT = 512                    # samples per tile (= fp32 matmul moving-dim max)
TILES = 17                  # tiles per core; 8*17=136 >= worst case 134
CAP = NT * TILES            # samples per core incl. padding

# "f32r": fp32 storage, fp32r (tf32-like full-rate) matmuls.
# "bf16": bf16 storage/matmuls (half the DMA), fp32 accumulate.
COMPUTE_MODE = os.environ.get("KERNEL_DT", "f32r")

_cached = {}


def _build():
    """Build + finalize the SPMD Bacc program (identical on all 8 cores)."""
    import concourse.bass as bass
    from concourse import bacc, mybir
    import concourse.tile as tile
    from concourse.bass import ts

    f32 = mybir.dt.float32
    bf16 = mybir.dt.bfloat16
    # storage dtype for everything the matmul touches. fp32r runs the PE at
    # full rate (vs 1/4 for fp32) with tf32-like precision; walrus requires
    # the whole producer chain (DMA, DVE) to carry the fp32r dtype, so the
    # tensors are declared fp32r end-to-end (numpy side is plain float32).
    sdt = bf16 if COMPUTE_MODE == "bf16" else mybir.dt.float32r

    def mmcast(ap):
        return ap

    add = mybir.AluOpType.add
    mx = mybir.AluOpType.max
    Sig = mybir.ActivationFunctionType.Sigmoid

    nc = bacc.Bacc(None, target_bir_lowering=False)

    # --- I/O ---------------------------------------------------------------
    xT = nc.declare_dram_parameter("xT", [E, CAP], sdt, isOutput=False)
    sp = nc.declare_dram_parameter("sp", [1, CAP], sdt, isOutput=False)
    tW1 = nc.declare_dram_parameter("tW1", [TILES, D, H], sdt, isOutput=False)
    tb1 = nc.declare_dram_parameter("tb1", [TILES, H], f32, isOutput=False)
    tW2 = nc.declare_dram_parameter("tW2", [TILES, H, H], sdt, isOutput=False)
    tb2 = nc.declare_dram_parameter("tb2", [TILES, H], f32, isOutput=False)
    tW3 = nc.declare_dram_parameter("tW3", [TILES, H, 3], sdt, isOutput=False)
    tb3 = nc.declare_dram_parameter("tb3", [TILES, 3, 1], f32, isOutput=False)
    siW1 = nc.declare_dram_parameter("siW1", [1, H], sdt, isOutput=False)
    sib1 = nc.declare_dram_parameter("sib1", [H], f32, isOutput=False)
    siW2 = nc.declare_dram_parameter("siW2", [H, SL], sdt, isOutput=False)
    sib2 = nc.declare_dram_parameter("sib2", [SL], f32, isOutput=False)
    soW1 = nc.declare_dram_parameter("soW1", [D, H], sdt, isOutput=False)
    sob1 = nc.declare_dram_parameter("sob1", [H], f32, isOutput=False)
    soW2 = nc.declare_dram_parameter("soW2", [H, H], sdt, isOutput=False)
    sob2 = nc.declare_dram_parameter("sob2", [H], f32, isOutput=False)
    soW3 = nc.declare_dram_parameter("soW3", [H, 1], sdt, isOutput=False)
    sob3 = nc.declare_dram_parameter("sob3", [1, 1], f32, isOutput=False)
    ctrl = nc.declare_dram_parameter("ctrl", [3, CAP], f32, isOutput=True)
    spd = nc.declare_dram_parameter("spd", [1, CAP], f32, isOutput=True)

    KD = D // 128   # 5 K-tiles of the branch/speed-head input dim
    KE = E // 128   # 4 of them come straight from the embedding
    KH = H // 128   # 2 K-tiles of the hidden dim
    MH = H // 128   # 2 M-tiles of the hidden dim

    # DRAM views with the 128-partition split made explicit.
    xT_r = xT[:].rearrange("(k p) b -> p k b", p=128)
    tW1_r = tW1[:].rearrange("t (k p) h -> t p k h", p=128)
    tW2_r = tW2[:].rearrange("t (k p) h -> t p k h", p=128)
    tW3_r = tW3[:].rearrange("t (k p) o -> t p k o", p=128)
    tb1_r = tb1[:].rearrange("t (m p) -> t p m", p=128)
    tb2_r = tb2[:].rearrange("t (m p) -> t p m", p=128)
    sib1_r = sib1[:].rearrange("(m p) -> p m", p=128)
    sib2_r = sib2[:].rearrange("(m p) -> p m", p=128)
    siW2_r = siW2[:].rearrange("(k p) m -> p k m", p=128)
    soW1_r = soW1[:].rearrange("(k p) h -> p k h", p=128)
    sob1_r = sob1[:].rearrange("(m p) -> p m", p=128)
    soW2_r = soW2[:].rearrange("(k p) h -> p k h", p=128)
    sob2_r = sob2[:].rearrange("(m p) -> p m", p=128)
    soW3_r = soW3[:].rearrange("(k p) m -> p k m", p=128)

    with tile.TileContext(nc) as tc:
        with (
            tc.tile_pool(name="fixed", bufs=1) as fixed,
            tc.tile_pool(name="wts", bufs=3) as wts,
            tc.tile_pool(name="xin", bufs=3) as xin,
            tc.tile_pool(name="acts", bufs=2) as acts,
            tc.tile_pool(name="outs", bufs=3) as outs,
            tc.tile_pool(name="psum", bufs=6, space="PSUM") as psum,
        ):
            # Shared (command-independent) weights: load once.
            siW1_s = fixed.tile([1, H], sdt)
            nc.sync.dma_start(out=siW1_s[:], in_=siW1[:])
            sib1_s = fixed.tile([128, 2], f32)
            nc.sync.dma_start(out=sib1_s[:], in_=sib1_r)
            siW2_s = fixed.tile([128, KH, SL], sdt)
            nc.sync.dma_start(out=siW2_s[:], in_=siW2_r)
            sib2_s = fixed.tile([128, 1], f32)
            nc.sync.dma_start(out=sib2_s[:], in_=sib2_r)
            soW1_s = fixed.tile([128, KD, H], sdt)
            nc.sync.dma_start(out=soW1_s[:], in_=soW1_r)
            sob1_s = fixed.tile([128, 2], f32)
            nc.sync.dma_start(out=sob1_s[:], in_=sob1_r)
            soW2_s = fixed.tile([128, KH, H], sdt)
            nc.sync.dma_start(out=soW2_s[:], in_=soW2_r)
            sob2_s = fixed.tile([128, 2], f32)
            nc.sync.dma_start(out=sob2_s[:], in_=sob2_r)
            soW3_s = fixed.tile([128, KH, 1], sdt)
            nc.sync.dma_start(out=soW3_s[:], in_=soW3_r)
            sob3_s = fixed.tile([1, 1], f32)
            nc.sync.dma_start(out=sob3_s[:], in_=sob3[:])

            for t in range(TILES):
                bsl = ts(t, NT)

                xt = xin.tile([128, KE, NT], sdt, tag="xt")
                nc.sync.dma_start(out=xt[:], in_=xT_r[:, :, bsl])
                spt = xin.tile([1, NT], sdt, tag="spt")
                nc.sync.dma_start(out=spt[:], in_=sp[0:1, bsl])

                w1t = wts.tile([128, KD, H], sdt, tag="w1")
                nc.sync.dma_start(out=w1t[:], in_=tW1_r[t])
                w2t = wts.tile([128, KH, H], sdt, tag="w2")
                nc.sync.dma_start(out=w2t[:], in_=tW2_r[t])
                w3t = wts.tile([128, KH, 3], sdt, tag="w3")
                nc.sync.dma_start(out=w3t[:], in_=tW3_r[t])
                b1t = wts.tile([128, 2], f32, tag="b1")
                nc.sync.dma_start(out=b1t[:], in_=tb1_r[t])
                b2t = wts.tile([128, 2], f32, tag="b2")
                nc.sync.dma_start(out=b2t[:], in_=tb2_r[t])
                b3t = wts.tile([3, 1], f32, tag="b3")
                nc.sync.dma_start(out=b3t[:], in_=tb3[t])

                # speed_in L1: h_T[j, b] = relu(si_W1[0,j]*speed[b] + si_b1[j])
                hst = acts.tile([128, 2, NT], sdt, tag="h")
                for m in range(2):
                    ph = psum.tile([128, NT], f32, tag="ps")
                    nc.tensor.matmul(
                        ph[:],
                        lhsT=mmcast(siW1_s[0:1, ts(m, 128)]),
                        rhs=mmcast(spt[:]),
                        start=True,
                        stop=True,
                    )
                    nc.vector.tensor_scalar(
                        hst[:, m, :], ph[:], sib1_s[:, m : m + 1], 0.0, add, mx
                    )
                # speed_in L2 (no relu): s_T[l, b]
                ps = psum.tile([128, NT], f32, tag="ps")
                for k in range(KH):
                    nc.tensor.matmul(
                        ps[:],
                        lhsT=mmcast(siW2_s[:, k, :]),
                        rhs=mmcast(hst[:, k, :]),
                        start=(k == 0),
                        stop=(k == KH - 1),
                    )
                sst = acts.tile([128, NT], sdt, tag="s")
                nc.vector.tensor_scalar(
                    sst[:], ps[:], sib2_s[:, 0:1], None, add
                )

                # emb_T K-tiles: 4 from the embedding + the computed s.
                rhsK = [xt[:, k, :] for k in range(KE)] + [sst[:]]

                # selected branch: L1 -> L2 -> L3+sigmoid
                a1 = acts.tile([128, 2, NT], sdt, tag="a1")
                for m in range(MH):
                    p1 = psum.tile([128, NT], f32, tag="ps")
                    for k in range(KD):
                        nc.tensor.matmul(
                            p1[:],
                            lhsT=mmcast(w1t[:, k, ts(m, 128)]),
                            rhs=mmcast(rhsK[k]),
                            start=(k == 0),
                            stop=(k == KD - 1),
                        )
                    nc.vector.tensor_scalar(
                        a1[:, m, :], p1[:], b1t[:, m : m + 1], 0.0, add, mx
                    )
                a2 = acts.tile([128, 2, NT], sdt, tag="a2")
                for m in range(MH):
                    p2 = psum.tile([128, NT], f32, tag="ps")
                    for k in range(KH):
                        nc.tensor.matmul(
                            p2[:],
                            lhsT=mmcast(w2t[:, k, ts(m, 128)]),
                            rhs=mmcast(a1[:, k, :]),
                            start=(k == 0),
                            stop=(k == KH - 1),
                        )
                    nc.vector.tensor_scalar(
                        a2[:, m, :], p2[:], b2t[:, m : m + 1], 0.0, add, mx
                    )
                p3 = psum.tile([3, NT], f32, tag="ps")
                for k in range(KH):
                    nc.tensor.matmul(
                        p3[:],
                        lhsT=mmcast(w3t[:, k, :]),
                        rhs=mmcast(a2[:, k, :]),
                        start=(k == 0),
                        stop=(k == KH - 1),
                    )
                cout = outs.tile([3, NT], f32, tag="c")
                nc.scalar.activation(cout[:], p3[:], Sig, bias=b3t[:, 0:1])
                nc.sync.dma_start(out=ctrl[:, bsl], in_=cout[:])

                # speed head: L1 -> L2 -> L3 (no activation on L3)
                q1 = acts.tile([128, 2, NT], sdt, tag="q1")
                for m in range(MH):
                    p4 = psum.tile([128, NT], f32, tag="ps")
                    for k in range(KD):
                        nc.tensor.matmul(
                            p4[:],
                            lhsT=mmcast(soW1_s[:, k, ts(m, 128)]),
                            rhs=mmcast(rhsK[k]),
                            start=(k == 0),
                            stop=(k == KD - 1),
                        )
                    nc.vector.tensor_scalar(
                        q1[:, m, :], p4[:], sob1_s[:, m : m + 1], 0.0, add, mx
                    )
                q2 = acts.tile([128, 2, NT], sdt, tag="q2")
                for m in range(MH):
                    p5 = psum.tile([128, NT], f32, tag="ps")
                    for k in range(KH):
                        nc.tensor.matmul(
                            p5[:],
                            lhsT=mmcast(soW2_s[:, k, ts(m, 128)]),
                            rhs=mmcast(q1[:, k, :]),
                            start=(k == 0),
                            stop=(k == KH - 1),
                        )
                    nc.vector.tensor_scalar(
                        q2[:, m, :], p5[:], sob2_s[:, m : m + 1], 0.0, add, mx
                    )
                p6 = psum.tile([1, NT], f32, tag="ps")
                for k in range(KH):
                    nc.tensor.matmul(
                        p6[:],
                        lhsT=mmcast(soW3_s[:, k, :]),
                        rhs=mmcast(q2[:, k, :]),
                        start=(k == 0),
                        stop=(k == KH - 1),
                    )
                sout = outs.tile([1, NT], f32, tag="sv")
                nc.vector.tensor_scalar(
                    sout[:], p6[:], sob3_s[0:1, 0:1], None, add
                )
                nc.sync.dma_start(out=spd[0:1, bsl], in_=sout[:])

    nc.finalize()
    return nc


def _get_nc():
    if "nc" not in _cached:
        _cached["nc"] = _build()
    return _cached["nc"]


def _route(command):
    """Bucket samples by command into single-command tiles of NT.

    Returns (slots, tile_cmd, nreal): slots[NCORES*CAP] maps device slot ->
    original sample index (bucket tails padded with a repeated in-bucket
    sample; trailing dummy tiles use sample 0), tile_cmd[NCORES*TILES] gives
    each tile's branch id, and slots[nreal:] are dummy-tile slots whose
    outputs must not be scattered back.
    """
    cmd = np.clip(np.asarray(command).astype(np.int64) - 1, 0, NB - 1)
    order = np.argsort(cmd, kind="stable")
    counts = np.bincount(cmd, minlength=NB)
    pieces = []
    tile_cmd = []
    pos = 0
    for c in range(NB):
        n = int(counts[c])
        if n == 0:
            continue
        idxs = order[pos : pos + n]
        pos += n
        ntile = -(-n // NT)
        pad = ntile * NT - n
        if pad:
            idxs = np.concatenate([idxs, np.full(pad, idxs[0], np.int64)])
        pieces.append(idxs)
        tile_cmd.extend([c] * ntile)
    nreal = len(tile_cmd) * NT
    ndum = NCORES * TILES - len(tile_cmd)
    assert ndum >= 0, "tile budget exceeded"
    if ndum:
        pieces.append(np.zeros(ndum * NT, np.int64))
        tile_cmd.extend([0] * ndum)
    return np.concatenate(pieces), np.asarray(tile_cmd), nreal


def _prep_in_maps(inputs, slots, tile_cmd):
    import ml_dtypes

    sdt_np = ml_dtypes.bfloat16 if COMPUTE_MODE == "bf16" else np.float32

    def s(x):  # storage-dtype cast
        return np.ascontiguousarray(np.asarray(x, np.float32).astype(sdt_np))

    def f(x):  # always-f32 (biases)
        return np.ascontiguousarray(np.asarray(x, np.float32))

    emb = np.asarray(inputs["embedding"], np.float32)
    speed = np.asarray(inputs["speed"], np.float32)

    gx = emb[slots]                                  # [NCORES*CAP, E]
    gs = speed[slots, 0]                             # [NCORES*CAP]

    shared = {
        "siW1": s(inputs["si_W1"].reshape(1, H)),
        "sib1": f(inputs["si_b1"]),
        "siW2": s(inputs["si_W2"]),
        "sib2": f(inputs["si_b2"]),
        "soW1": s(inputs["so_W1"]),
        "sob1": f(inputs["so_b1"]),
        "soW2": s(inputs["so_W2"]),
        "sob2": f(inputs["so_b2"]),
        "soW3": s(inputs["so_W3"]),
        "sob3": f(inputs["so_b3"].reshape(1, 1)),
    }
    bW1 = np.asarray(inputs["bW1"], np.float32)
    bb1 = np.asarray(inputs["bb1"], np.float32)
    bW2 = np.asarray(inputs["bW2"], np.float32)
    bb2 = np.asarray(inputs["bb2"], np.float32)
    bW3 = np.asarray(inputs["bW3"], np.float32)
    bb3 = np.asarray(inputs["bb3"], np.float32)

    in_maps = []
    for c in range(NCORES):
        sl = slice(c * CAP, (c + 1) * CAP)
        tc_ = tile_cmd[c * TILES : (c + 1) * TILES]
        in_map = {
            "xT": s(gx[sl].T),
            "sp": s(gs[sl].reshape(1, CAP)),
            "tW1": s(bW1[tc_]),
            "tb1": f(bb1[tc_]),
            "tW2": s(bW2[tc_]),
            "tb2": f(bb2[tc_]),
            "tW3": s(bW3[tc_]),
            "tb3": f(bb3[tc_].reshape(TILES, 3, 1)),
        }
        in_map.update(shared)
        in_maps.append(in_map)
    return in_maps


def _run(inputs, trace=False):
    from concourse.bass_utils import run_bass_kernel_spmd

    command = np.asarray(inputs["command"])
    assert command.shape == (B,), command.shape
    slots, tile_cmd, nreal = _route(command)
    in_maps = _prep_in_maps(inputs, slots, tile_cmd)

    nc = _get_nc()
    res = run_bass_kernel_spmd(nc, in_maps, list(range(NCORES)), trace=trace)

    control = np.zeros((B, 3), np.float32)
    speed_pred = np.zeros((B, 1), np.float32)
    for c in range(NCORES):
        lo = c * CAP
        nvalid = min(max(nreal - lo, 0), CAP)
        if nvalid == 0:
            continue
        sl = slots[lo : lo + nvalid]
        control[sl] = res.results[c]["ctrl"][:, :nvalid].T
        speed_pred[sl, 0] = res.results[c]["spd"][0, :nvalid]

    # Out-of-range commands select no branch in the reference -> zeros.
    bad = (command < 1) | (command > NB)
    if bad.any():
        control[bad] = 0.0
    return control, speed_pred, res


def kernel(**inputs):
    control, speed_pred, _ = _run(inputs, trace=False)
    return control, speed_pred


# revision 4
# speedup vs baseline: 1.1841x; 1.1841x over previous
"""Trainium2 Bass kernel for nn_CILRSVAEModel (CILRS-style MoE branch routing).

Model (per sample):
  s      = relu(speed @ si_W1 + si_b1) @ si_W2 + si_b2            # [SL]
  emb    = concat(embedding, s)                                   # [D=640]
  branch = command-selected MLP: sigmoid(relu(relu(emb@W1+b1)@W2+b2)@W3+b3)
  speed_pred = relu(relu(emb@so_W1+so_b1)@so_W2+so_b2)@so_W3+so_b3

Strategy: pure data parallel over batch, plus host-side MoE routing — samples
are bucketed by `command` into single-command tiles of 512, so the device
computes only the selected branch per sample (6x fewer branch FLOPs than the
dense reference). Each tile's branch weights are gathered host-side into a
per-tile packed weight image, keeping the device program fully static. All
device-facing arrays are packed host-side into [128-partition, contiguous]
layouts so every DMA is 128 descriptors of multi-KB contiguous runs, and the
embedding is pre-transposed so activations live in [feature, batch] layout
end-to-end (no on-device transposes). Matmuls run as lhsT=[K,M] weights x
rhs=[K,N] activations with N=512 batch columns, in fp32r (full PE rate).
PSUM->SBUF bias+activation drains are split between the Scalar (ACT) and
Vector (DVE) engines; relu/sigmoid/identity share one ACT table set so there
are no table reloads.
"""

import os

# The device program runs through jax's axon/PJRT backend; make sure a
# harness-set JAX_PLATFORMS=cpu doesn't hide the neuron devices.
if os.environ.get("JAX_PLATFORMS", None) in ("cpu",):
    os.environ.pop("JAX_PLATFORMS")

import sys

import numpy as np

for _p in ("/opt/trn_rl_repo",):
    if _p not in sys.path and os.path.isdir(_p):
        sys.path.insert(0, _p)

B = 65536
E = 512
SL = 128
H = 256
NB = 6
D = E + SL
NCORES = 8
NT = 512                    # samples per tile (= fp32 matmul moving-dim max)
TILES = 17                  # tiles per core; 8*17=136 >= worst case 134
CAP = NT * TILES            # samples per core incl. padding

KD = D // 128               # 5 K-tiles of the branch input dim
KE = E // 128               # 4 of them come straight from the embedding
KH = H // 128               # 2 K-tiles of the hidden dim
MH = H // 128               # 2 M-tiles of the hidden dim

# packed per-tile weight image: per partition p (=row within a K-tile):
#   [ W1 (KD*H) | W2 (KH*H) | W3 (KH*3) ]
W2OFF = KD * H              # 1280
W3OFF = W2OFF + KH * H      # 1792
WCOLS = W3OFF + KH * 3      # 1798

# "f32r": fp32 storage, fp32r (tf32-like full-rate) matmuls.
# "bf16": bf16 storage/matmuls (half the DMA), fp32 accumulate.
COMPUTE_MODE = os.environ.get("KERNEL_DT", "f32r")

_cached = {}


def _build():
    """Build + finalize the SPMD Bacc program (identical on all 8 cores)."""
    from concourse import bacc, mybir
    import concourse.tile as tile
    from concourse.bass import ts

    f32 = mybir.dt.float32
    bf16 = mybir.dt.bfloat16
    # storage dtype for everything the matmul touches. fp32r runs the PE at
    # full rate (vs 1/4 for fp32) with tf32-like precision; walrus requires
    # the whole producer chain (DMA, DVE) to carry the fp32r dtype, so the
    # tensors are declared fp32r end-to-end (numpy side is plain float32).
    sdt = bf16 if COMPUTE_MODE == "bf16" else mybir.dt.float32r

    add = mybir.AluOpType.add
    mx = mybir.AluOpType.max
    Relu = mybir.ActivationFunctionType.Relu
    Ident = mybir.ActivationFunctionType.Identity
    Sig = mybir.ActivationFunctionType.Sigmoid

    nc = bacc.Bacc(None, target_bir_lowering=False)

    # --- I/O ---------------------------------------------------------------
    xh = nc.declare_dram_parameter("xh", [TILES, 128, KE, NT], sdt, isOutput=False)
    sp = nc.declare_dram_parameter("sp", [1, CAP], sdt, isOutput=False)
    wh = nc.declare_dram_parameter("wh", [TILES, 128, WCOLS], sdt, isOutput=False)
    bh = nc.declare_dram_parameter("bh", [TILES, 128, 5], f32, isOutput=False)
    siW1 = nc.declare_dram_parameter("siW1", [1, H], sdt, isOutput=False)
    sib1 = nc.declare_dram_parameter("sib1", [H], f32, isOutput=False)
    siW2 = nc.declare_dram_parameter("siW2", [H, SL], sdt, isOutput=False)
    sib2 = nc.declare_dram_parameter("sib2", [SL], f32, isOutput=False)
    soW1 = nc.declare_dram_parameter("soW1", [D, H], sdt, isOutput=False)
    sob1 = nc.declare_dram_parameter("sob1", [H], f32, isOutput=False)
    soW2 = nc.declare_dram_parameter("soW2", [H, H], sdt, isOutput=False)
    sob2 = nc.declare_dram_parameter("sob2", [H], f32, isOutput=False)
    soW3 = nc.declare_dram_parameter("soW3", [H, 1], sdt, isOutput=False)
    sob3 = nc.declare_dram_parameter("sob3", [1, 1], f32, isOutput=False)
    ctrl = nc.declare_dram_parameter("ctrl", [3, CAP], f32, isOutput=True)
    spd = nc.declare_dram_parameter("spd", [1, CAP], f32, isOutput=True)

    sib1_r = sib1[:].rearrange("(m p) -> p m", p=128)
    sib2_r = sib2[:].rearrange("(m p) -> p m", p=128)
    siW2_r = siW2[:].rearrange("(k p) m -> p k m", p=128)
    soW1_r = soW1[:].rearrange("(k p) h -> p k h", p=128)
    sob1_r = sob1[:].rearrange("(m p) -> p m", p=128)
    soW2_r = soW2[:].rearrange("(k p) h -> p k h", p=128)
    sob2_r = sob2[:].rearrange("(m p) -> p m", p=128)
    soW3_r = soW3[:].rearrange("(k p) m -> p k m", p=128)

    with tile.TileContext(nc) as tc:
        with (
            tc.tile_pool(name="fixed", bufs=1) as fixed,
            tc.tile_pool(name="wts", bufs=3) as wts,
            tc.tile_pool(name="xin", bufs=3) as xin,
            tc.tile_pool(name="acts", bufs=2) as acts,
            tc.tile_pool(name="outs", bufs=3) as outs,
            tc.tile_pool(name="psum", bufs=7, space="PSUM") as psum,
        ):
            # Shared (command-independent) weights: load once.
            siW1_s = fixed.tile([1, H], sdt)
            nc.sync.dma_start(out=siW1_s[:], in_=siW1[:])
            sib1_s = fixed.tile([128, 2], f32)
            nc.sync.dma_start(out=sib1_s[:], in_=sib1_r)
            siW2_s = fixed.tile([128, KH, SL], sdt)
            nc.sync.dma_start(out=siW2_s[:], in_=siW2_r)
            sib2_s = fixed.tile([128, 1], f32)
            nc.sync.dma_start(out=sib2_s[:], in_=sib2_r)
            soW1_s = fixed.tile([128, KD, H], sdt)
            nc.sync.dma_start(out=soW1_s[:], in_=soW1_r)
            sob1_s = fixed.tile([128, 2], f32)
            nc.sync.dma_start(out=sob1_s[:], in_=sob1_r)
            soW2_s = fixed.tile([128, KH, H], sdt)
            nc.sync.dma_start(out=soW2_s[:], in_=soW2_r)
            sob2_s = fixed.tile([128, 2], f32)
            nc.sync.dma_start(out=sob2_s[:], in_=sob2_r)
            soW3_s = fixed.tile([128, KH, 1], sdt)
            nc.sync.dma_start(out=soW3_s[:], in_=soW3_r)
            sob3_s = fixed.tile([1, 1], f32)
            nc.sync.dma_start(out=sob3_s[:], in_=sob3[:])

            for t in range(TILES):
                bsl = ts(t, NT)

                xt = xin.tile([128, KE, NT], sdt, tag="xt")
                nc.sync.dma_start(out=xt[:], in_=xh[t])
                spt = xin.tile([1, NT], sdt, tag="spt")
                nc.sync.dma_start(out=spt[:], in_=sp[0:1, bsl])
                wht = wts.tile([128, WCOLS], sdt, tag="w")
                nc.sync.dma_start(out=wht[:], in_=wh[t])
                bht = wts.tile([128, 5], f32, tag="b")
                nc.sync.dma_start(out=bht[:], in_=bh[t])

                def w1(k, m):
                    return wht[:, k * H + m * 128 : k * H + (m + 1) * 128]

                def w2(k, m):
                    return wht[:, W2OFF + k * H + m * 128 : W2OFF + k * H + (m + 1) * 128]

                def w3(k):
                    return wht[:, W3OFF + k * 3 : W3OFF + (k + 1) * 3]

                # speed_in L1: h_T[j,b] = relu(si_W1[0,j]*speed[b] + si_b1[j])
                hst = acts.tile([128, 2, NT], sdt, tag="h")
                ph = [psum.tile([128, NT], f32, tag="ps", name=f"ph{t}_{i}") for i in range(2)]
                for m in range(2):
                    nc.tensor.matmul(
                        ph[m][:],
                        lhsT=siW1_s[0:1, ts(m, 128)],
                        rhs=spt[:],
                        start=True,
                        stop=True,
                    )
                for m in range(2):
                    nc.scalar.activation(
                        hst[:, m, :], ph[m][:], Relu, bias=sib1_s[:, m : m + 1]
                    )

                # branch L1 and speed-head L1 over the embedding K-tiles
                # (k=0..3); the 5th K-tile (s) joins once computed. Keeping
                # these groups open lets the PE run embedding matmuls while
                # ACT drains the speed_in chain.
                p1 = [psum.tile([128, NT], f32, tag="ps", name=f"p1_{t}_{i}") for i in range(MH)]
                p4 = [psum.tile([128, NT], f32, tag="ps", name=f"p4_{t}_{i}") for i in range(MH)]
                for m in range(MH):
                    for k in range(KE):
                        nc.tensor.matmul(
                            p1[m][:],
                            lhsT=w1(k, m),
                            rhs=xt[:, k, :],
                            start=(k == 0),
                            stop=False,
                        )
                for m in range(MH):
                    for k in range(KE):
                        nc.tensor.matmul(
                            p4[m][:],
                            lhsT=soW1_s[:, k, ts(m, 128)],
                            rhs=xt[:, k, :],
                            start=(k == 0),
                            stop=False,
                        )

                # speed_in L2 (no relu): s_T[l,b]
                ps = psum.tile([128, NT], f32, tag="ps")
                for k in range(KH):
                    nc.tensor.matmul(
                        ps[:],
                        lhsT=siW2_s[:, k, :],
                        rhs=hst[:, k, :],
                        start=(k == 0),
                        stop=(k == KH - 1),
                    )
                sst = acts.tile([128, NT], sdt, tag="s")
                nc.scalar.activation(sst[:], ps[:], Ident, bias=sib2_s[:, 0:1])

                # close the L1 groups with the s K-tile
                for m in range(MH):
                    nc.tensor.matmul(
                        p1[m][:], lhsT=w1(KE, m), rhs=sst[:], start=False, stop=True
                    )
                for m in range(MH):
                    nc.tensor.matmul(
                        p4[m][:],
                        lhsT=soW1_s[:, KE, ts(m, 128)],
                        rhs=sst[:],
                        start=False,
                        stop=True,
                    )

                a1 = acts.tile([128, 2, NT], sdt, tag="a1")
                q1 = acts.tile([128, 2, NT], sdt, tag="q1")
                for m in range(MH):
                    nc.vector.tensor_scalar(
                        a1[:, m, :], p1[m][:], bht[:, m : m + 1], 0.0, add, mx
                    )
                for m in range(MH):
                    nc.vector.tensor_scalar(
                        q1[:, m, :], p4[m][:], sob1_s[:, m : m + 1], 0.0, add, mx
                    )

                # L2 for both heads
                a2 = acts.tile([128, 2, NT], sdt, tag="a2")
                q2 = acts.tile([128, 2, NT], sdt, tag="q2")
                p2 = [psum.tile([128, NT], f32, tag="ps", name=f"p2_{t}_{i}") for i in range(MH)]
                p5 = [psum.tile([128, NT], f32, tag="ps", name=f"p5_{t}_{i}") for i in range(MH)]
                for m in range(MH):
                    for k in range(KH):
                        nc.tensor.matmul(
                            p2[m][:],
                            lhsT=w2(k, m),
                            rhs=a1[:, k, :],
                            start=(k == 0),
                            stop=(k == KH - 1),
                        )
                for m in range(MH):
                    for k in range(KH):
                        nc.tensor.matmul(
                            p5[m][:],
                            lhsT=soW2_s[:, k, ts(m, 128)],
                            rhs=q1[:, k, :],
                            start=(k == 0),
                            stop=(k == KH - 1),
                        )
                for m in range(MH):
                    nc.vector.tensor_scalar(
                        a2[:, m, :], p2[m][:], bht[:, 2 + m : 3 + m], 0.0, add, mx
                    )
                for m in range(MH):
                    nc.vector.tensor_scalar(
                        q2[:, m, :], p5[m][:], sob2_s[:, m : m + 1], 0.0, add, mx
                    )

                # L3: branch logits -> sigmoid; speed head -> identity
                p3 = psum.tile([3, NT], f32, tag="ps")
                for k in range(KH):
                    nc.tensor.matmul(
                        p3[:],
                        lhsT=w3(k),
                        rhs=a2[:, k, :],
                        start=(k == 0),
                        stop=(k == KH - 1),
                    )
                cout = outs.tile([3, NT], f32, tag="c")
                nc.scalar.activation(cout[:], p3[:], Sig, bias=bht[0:3, 4:5])
                nc.sync.dma_start(out=ctrl[:, bsl], in_=cout[:])

                p6 = psum.tile([1, NT], f32, tag="ps")
                for k in range(KH):
                    nc.tensor.matmul(
                        p6[:],
                        lhsT=soW3_s[:, k, :],
                        rhs=q2[:, k, :],
                        start=(k == 0),
                        stop=(k == KH - 1),
                    )
                sout = outs.tile([1, NT], f32, tag="sv")
                nc.scalar.activation(sout[:], p6[:], Ident, bias=sob3_s[0:1, 0:1])
                nc.sync.dma_start(out=spd[0:1, bsl], in_=sout[:])

    nc.finalize()
    return nc


def _get_nc():
    if "nc" not in _cached:
        _cached["nc"] = _build()
    return _cached["nc"]


def _route(command):
    """Bucket samples by command into single-command tiles of NT.

    Returns (slots, tile_cmd, nreal): slots[NCORES*CAP] maps device slot ->
    original sample index (bucket tails padded with a repeated in-bucket
    sample; trailing dummy tiles use sample 0), tile_cmd[NCORES*TILES] gives
    each tile's branch id, and slots[nreal:] are dummy-tile slots whose
    outputs must not be scattered back.
    """
    cmd = np.clip(np.asarray(command).astype(np.int64) - 1, 0, NB - 1)
    order = np.argsort(cmd, kind="stable")
    counts = np.bincount(cmd, minlength=NB)
    pieces = []
    tile_cmd = []
    pos = 0
    for c in range(NB):
        n = int(counts[c])
        if n == 0:
            continue
        idxs = order[pos : pos + n]
        pos += n
        ntile = -(-n // NT)
        pad = ntile * NT - n
        if pad:
            idxs = np.concatenate([idxs, np.full(pad, idxs[0], np.int64)])
        pieces.append(idxs)
        tile_cmd.extend([c] * ntile)
    nreal = len(tile_cmd) * NT
    ndum = NCORES * TILES - len(tile_cmd)
    assert ndum >= 0, "tile budget exceeded"
    if ndum:
        pieces.append(np.zeros(ndum * NT, np.int64))
        tile_cmd.extend([0] * ndum)
    return np.concatenate(pieces), np.asarray(tile_cmd), nreal


def _prep_in_maps(inputs, slots, tile_cmd):
    import ml_dtypes

    sdt_np = ml_dtypes.bfloat16 if COMPUTE_MODE == "bf16" else np.float32

    def s(x):  # storage-dtype cast
        return np.ascontiguousarray(np.asarray(x, np.float32).astype(sdt_np))

    def f(x):  # always-f32 (biases)
        return np.ascontiguousarray(np.asarray(x, np.float32))

    emb = np.asarray(inputs["embedding"], np.float32)
    speed = np.asarray(inputs["speed"], np.float32)

    gx = emb[slots]                                  # [NCORES*CAP, E]
    gs = speed[slots, 0]                             # [NCORES*CAP]

    # packed per-command weight/bias images (gathered per tile below)
    bW1 = np.asarray(inputs["bW1"], np.float32)
    bb1 = np.asarray(inputs["bb1"], np.float32)
    bW2 = np.asarray(inputs["bW2"], np.float32)
    bb2 = np.asarray(inputs["bb2"], np.float32)
    bW3 = np.asarray(inputs["bW3"], np.float32)
    bb3 = np.asarray(inputs["bb3"], np.float32)
    pw = np.empty((NB, 128, WCOLS), np.float32)
    pb = np.zeros((NB, 128, 5), np.float32)
    for c in range(NB):
        w1 = bW1[c].reshape(KD, 128, H).transpose(1, 0, 2).reshape(128, KD * H)
        w2 = bW2[c].reshape(KH, 128, H).transpose(1, 0, 2).reshape(128, KH * H)
        w3 = bW3[c].reshape(KH, 128, 3).transpose(1, 0, 2).reshape(128, KH * 3)
        pw[c] = np.concatenate([w1, w2, w3], axis=1)
        pb[c, :, 0:2] = bb1[c].reshape(2, 128).T
        pb[c, :, 2:4] = bb2[c].reshape(2, 128).T
        pb[c, 0:3, 4] = bb3[c]
    pw = pw.astype(sdt_np)

    shared = {
        "siW1": s(inputs["si_W1"].reshape(1, H)),
        "sib1": f(inputs["si_b1"]),
        "siW2": s(inputs["si_W2"]),
        "sib2": f(inputs["si_b2"]),
        "soW1": s(inputs["so_W1"]),
        "sob1": f(inputs["so_b1"]),
        "soW2": s(inputs["so_W2"]),
        "sob2": f(inputs["so_b2"]),
        "soW3": s(inputs["so_W3"]),
        "sob3": f(inputs["so_b3"].reshape(1, 1)),
    }

    in_maps = []
    for c in range(NCORES):
        sl = slice(c * CAP, (c + 1) * CAP)
        tc_ = tile_cmd[c * TILES : (c + 1) * TILES]
        # [tile, partition(=dim within K-tile), K-tile, sample]
        xc = gx[sl].reshape(TILES, NT, KE, 128).transpose(0, 3, 2, 1)
        in_map = {
            "xh": s(xc),
            "sp": s(gs[sl].reshape(1, CAP)),
            "wh": np.ascontiguousarray(pw[tc_]),
            "bh": np.ascontiguousarray(pb[tc_]),
        }
        in_map.update(shared)
        in_maps.append(in_map)
    return in_maps


def _run(inputs, trace=False):
    from concourse.bass_utils import run_bass_kernel_spmd

    command = np.asarray(inputs["command"])
    assert command.shape == (B,), command.shape
    slots, tile_cmd, nreal = _route(command)
    in_maps = _prep_in_maps(inputs, slots, tile_cmd)

    nc = _get_nc()
    res = run_bass_kernel_spmd(nc, in_maps, list(range(NCORES)), trace=trace)

    control = np.zeros((B, 3), np.float32)
    speed_pred = np.zeros((B, 1), np.float32)
    for c in range(NCORES):
        lo = c * CAP
        nvalid = min(max(nreal - lo, 0), CAP)
        if nvalid == 0:
            continue
        sl = slots[lo : lo + nvalid]
        control[sl] = res.results[c]["ctrl"][:, :nvalid].T
        speed_pred[sl, 0] = res.results[c]["spd"][0, :nvalid]

    # Out-of-range commands select no branch in the reference -> zeros.
    bad = (command < 1) | (command > NB)
    if bad.any():
        control[bad] = 0.0
    return control, speed_pred, res


def kernel(**inputs):
    control, speed_pred, _ = _run(inputs, trace=False)
    return control, speed_pred


# revision 6
# speedup vs baseline: 1.2948x; 1.0935x over previous
"""Trainium2 Bass kernel for nn_CILRSVAEModel (CILRS-style MoE branch routing).

Model (per sample):
  s      = relu(speed @ si_W1 + si_b1) @ si_W2 + si_b2            # [SL]
  emb    = concat(embedding, s)                                   # [D=640]
  branch = command-selected MLP: sigmoid(relu(relu(emb@W1+b1)@W2+b2)@W3+b3)
  speed_pred = relu(relu(emb@so_W1+so_b1)@so_W2+so_b2)@so_W3+so_b3

Strategy: pure data parallel over batch, plus host-side MoE routing — samples
are bucketed by `command` into single-command tiles of 512, so the device
computes only the selected branch per sample (6x fewer branch FLOPs than the
dense reference). Each tile's branch weights are gathered host-side into a
per-tile packed weight image, keeping the device program fully static. All
device-facing arrays are packed host-side into [128-partition, contiguous]
layouts so every DMA is 128 descriptors of multi-KB contiguous runs, and the
embedding is pre-transposed so activations live in [feature, batch] layout
end-to-end (no on-device transposes). Matmuls run as lhsT=[K,M] weights x
rhs=[K,N] activations with N=512 batch columns, in fp32r (full PE rate).
PSUM->SBUF bias+activation drains are split between the Scalar (ACT) and
Vector (DVE) engines; relu/sigmoid/identity share one ACT table set so there
are no table reloads.
"""

import os

# The device program runs through jax's axon/PJRT backend; make sure a
# harness-set JAX_PLATFORMS=cpu doesn't hide the neuron devices.
if os.environ.get("JAX_PLATFORMS", None) in ("cpu",):
    os.environ.pop("JAX_PLATFORMS")

import sys

import numpy as np

for _p in ("/opt/trn_rl_repo",):
    if _p not in sys.path and os.path.isdir(_p):
        sys.path.insert(0, _p)

B = 65536
E = 512
SL = 128
H = 256
NB = 6
D = E + SL
NCORES = 8
NT = 512                    # samples per tile (= fp32 matmul moving-dim max)
TILES = 17                  # tiles per core; 8*17=136 >= worst case 134
CAP = NT * TILES            # samples per core incl. padding

KD = D // 128               # 5 K-tiles of the branch input dim
KE = E // 128               # 4 of them come straight from the embedding
KH = H // 128               # 2 K-tiles of the hidden dim
MH = H // 128               # 2 M-tiles of the hidden dim

# packed per-tile weight image: per partition p (=row within a K-tile):
#   [ W1 (KD*H) | W2 (KH*H) | W3 (KH*3) ]
W2OFF = KD * H              # 1280
W3OFF = W2OFF + KH * H      # 1792
WCOLS = W3OFF + KH * 3      # 1798

# "f32r": fp32 storage, fp32r (tf32-like full-rate) matmuls.
# "bf16": bf16 storage/matmuls (half the DMA), fp32 accumulate.
COMPUTE_MODE = os.environ.get("KERNEL_DT", "f32r")

_cached = {}


def _build():
    """Build + finalize the SPMD Bacc program (identical on all 8 cores)."""
    from concourse import bacc, mybir
    import concourse.tile as tile
    from concourse.bass import ts

    f32 = mybir.dt.float32
    bf16 = mybir.dt.bfloat16
    # storage dtype for everything the matmul touches. fp32r runs the PE at
    # full rate (vs 1/4 for fp32) with tf32-like precision; walrus requires
    # the whole producer chain (DMA, DVE) to carry the fp32r dtype, so the
    # tensors are declared fp32r end-to-end (numpy side is plain float32).
    sdt = bf16 if COMPUTE_MODE == "bf16" else mybir.dt.float32r

    add = mybir.AluOpType.add
    mx = mybir.AluOpType.max
    Relu = mybir.ActivationFunctionType.Relu
    Ident = mybir.ActivationFunctionType.Identity
    Sig = mybir.ActivationFunctionType.Sigmoid

    nc = bacc.Bacc(None, target_bir_lowering=False)

    # --- I/O ---------------------------------------------------------------
    xh = nc.declare_dram_parameter("xh", [TILES, 128, KE, NT], sdt, isOutput=False)
    sp = nc.declare_dram_parameter("sp", [1, CAP], sdt, isOutput=False)
    wh = nc.declare_dram_parameter("wh", [TILES, 128, WCOLS], sdt, isOutput=False)
    bh = nc.declare_dram_parameter("bh", [TILES, 128, 5], f32, isOutput=False)
    siW1 = nc.declare_dram_parameter("siW1", [1, H], sdt, isOutput=False)
    sib1 = nc.declare_dram_parameter("sib1", [H], f32, isOutput=False)
    siW2 = nc.declare_dram_parameter("siW2", [H, SL], sdt, isOutput=False)
    sib2 = nc.declare_dram_parameter("sib2", [SL], f32, isOutput=False)
    soW1 = nc.declare_dram_parameter("soW1", [D, H], sdt, isOutput=False)
    sob1 = nc.declare_dram_parameter("sob1", [H], f32, isOutput=False)
    soW2 = nc.declare_dram_parameter("soW2", [H, H], sdt, isOutput=False)
    sob2 = nc.declare_dram_parameter("sob2", [H], f32, isOutput=False)
    soW3 = nc.declare_dram_parameter("soW3", [H, 1], sdt, isOutput=False)
    sob3 = nc.declare_dram_parameter("sob3", [1, 1], f32, isOutput=False)
    ctrl = nc.declare_dram_parameter("ctrl", [3, CAP], f32, isOutput=True)
    spd = nc.declare_dram_parameter("spd", [1, CAP], f32, isOutput=True)

    sib1_r = sib1[:].rearrange("(m p) -> p m", p=128)
    sib2_r = sib2[:].rearrange("(m p) -> p m", p=128)
    siW2_r = siW2[:].rearrange("(k p) m -> p k m", p=128)
    soW1_r = soW1[:].rearrange("(k p) h -> p k h", p=128)
    sob1_r = sob1[:].rearrange("(m p) -> p m", p=128)
    soW2_r = soW2[:].rearrange("(k p) h -> p k h", p=128)
    sob2_r = sob2[:].rearrange("(m p) -> p m", p=128)
    soW3_r = soW3[:].rearrange("(k p) m -> p k m", p=128)

    with tile.TileContext(nc) as tc:
        with (
            tc.tile_pool(name="fixed", bufs=1) as fixed,
            tc.tile_pool(name="wts", bufs=3) as wts,
            tc.tile_pool(name="xin", bufs=3) as xin,
            tc.tile_pool(name="acts", bufs=2) as acts,
            tc.tile_pool(name="outs", bufs=3) as outs,
            tc.tile_pool(name="psum", bufs=8, space="PSUM") as psum,
        ):
            # Shared (command-independent) weights: load once.
            siW1_s = fixed.tile([1, H], sdt)
            nc.sync.dma_start(out=siW1_s[:], in_=siW1[:])
            sib1_s = fixed.tile([128, 2], f32)
            nc.sync.dma_start(out=sib1_s[:], in_=sib1_r)
            siW2_s = fixed.tile([128, KH, SL], sdt)
            nc.sync.dma_start(out=siW2_s[:], in_=siW2_r)
            sib2_s = fixed.tile([128, 1], f32)
            nc.sync.dma_start(out=sib2_s[:], in_=sib2_r)
            soW1_s = fixed.tile([128, KD, H], sdt)
            nc.sync.dma_start(out=soW1_s[:], in_=soW1_r)
            sob1_s = fixed.tile([128, 2], f32)
            nc.sync.dma_start(out=sob1_s[:], in_=sob1_r)
            soW2_s = fixed.tile([128, KH, H], sdt)
            nc.sync.dma_start(out=soW2_s[:], in_=soW2_r)
            sob2_s = fixed.tile([128, 2], f32)
            nc.sync.dma_start(out=sob2_s[:], in_=sob2_r)
            soW3_s = fixed.tile([128, KH, 1], sdt)
            nc.sync.dma_start(out=soW3_s[:], in_=soW3_r)
            sob3_s = fixed.tile([1, 1], f32)
            nc.sync.dma_start(out=sob3_s[:], in_=sob3[:])

            def load_inputs(t):
                """Issue the input DMAs for tile t (one tile of prefetch)."""
                xt = xin.tile([128, KE, NT], sdt, tag="xt", name=f"xt{t}")
                nc.sync.dma_start(out=xt[:], in_=xh[t])
                spt = xin.tile([1, NT], sdt, tag="spt", name=f"spt{t}")
                nc.sync.dma_start(out=spt[:], in_=sp[0:1, ts(t, NT)])
                wht = wts.tile([128, WCOLS], sdt, tag="w", name=f"wht{t}")
                nc.sync.dma_start(out=wht[:], in_=wh[t])
                bht = wts.tile([128, 5], f32, tag="b", name=f"bht{t}")
                nc.sync.dma_start(out=bht[:], in_=bh[t])
                return xt, spt, wht, bht

            def emit_ph(t, spt):
                """speed_in L1 matmuls for tile t (K=1 outer products)."""
                ph = [
                    psum.tile([128, NT], f32, tag="ps", name=f"ph{t}_{i}")
                    for i in range(2)
                ]
                for m in range(2):
                    nc.tensor.matmul(
                        ph[m][:],
                        lhsT=siW1_s[0:1, ts(m, 128)],
                        rhs=spt[:],
                        start=True,
                        stop=True,
                    )
                return ph

            inp = load_inputs(0)
            ph = emit_ph(0, inp[1])

            for t in range(TILES):
                bsl = ts(t, NT)
                xt, spt, wht, bht = inp

                def w1(k, m):
                    return wht[:, k * H + m * 128 : k * H + (m + 1) * 128]

                def w2(k, m):
                    return wht[:, W2OFF + k * H + m * 128 : W2OFF + k * H + (m + 1) * 128]

                def w3(k):
                    return wht[:, W3OFF + k * 3 : W3OFF + (k + 1) * 3]

                # prefetch next tile's inputs a full tile ahead
                if t + 1 < TILES:
                    inp = load_inputs(t + 1)

                # drain speed_in L1 (ACT) while the PE starts on this tile's
                # embedding matmuls
                hst = acts.tile([128, 2, NT], sdt, tag="h", name=f"hst{t}")
                for m in range(2):
                    nc.scalar.activation(
                        hst[:, m, :], ph[m][:], Relu, bias=sib1_s[:, m : m + 1]
                    )

                # branch L1 over the embedding K-tiles (k=0..3); the 5th
                # K-tile (s) joins once computed.
                p1 = [psum.tile([128, NT], f32, tag="ps", name=f"p1_{t}_{i}") for i in range(MH)]
                p4 = [psum.tile([128, NT], f32, tag="ps", name=f"p4_{t}_{i}") for i in range(MH)]
                for m in range(MH):
                    for k in range(KE):
                        nc.tensor.matmul(
                            p1[m][:],
                            lhsT=w1(k, m),
                            rhs=xt[:, k, :],
                            start=(k == 0),
                            stop=False,
                        )

                # speed_in L2 (no relu): s_T[l,b]
                ps = psum.tile([128, NT], f32, tag="ps", name=f"ps{t}")
                for k in range(KH):
                    nc.tensor.matmul(
                        ps[:],
                        lhsT=siW2_s[:, k, :],
                        rhs=hst[:, k, :],
                        start=(k == 0),
                        stop=(k == KH - 1),
                    )
                sst = acts.tile([128, NT], sdt, tag="s", name=f"sst{t}")
                nc.scalar.activation(sst[:], ps[:], Ident, bias=sib2_s[:, 0:1])

                # speed-head L1 embedding K-tiles keep the PE busy while ACT
                # produces s
                for m in range(MH):
                    for k in range(KE):
                        nc.tensor.matmul(
                            p4[m][:],
                            lhsT=soW1_s[:, k, ts(m, 128)],
                            rhs=xt[:, k, :],
                            start=(k == 0),
                            stop=False,
                        )

                # close the L1 groups with the s K-tile; drain each group the
                # moment its last matmul lands
                a1 = acts.tile([128, 2, NT], sdt, tag="a1", name=f"a1_{t}")
                q1 = acts.tile([128, 2, NT], sdt, tag="q1", name=f"q1_{t}")
                for m in range(MH):
                    nc.tensor.matmul(
                        p1[m][:], lhsT=w1(KE, m), rhs=sst[:], start=False, stop=True
                    )
                for m in range(MH):
                    nc.vector.tensor_scalar(
                        a1[:, m, :], p1[m][:], bht[:, m : m + 1], 0.0, add, mx
                    )
                for m in range(MH):
                    nc.tensor.matmul(
                        p4[m][:],
                        lhsT=soW1_s[:, KE, ts(m, 128)],
                        rhs=sst[:],
                        start=False,
                        stop=True,
                    )

                # next tile's speed_in L1 fills the PE while the DVE drains a1
                if t + 1 < TILES:
                    ph = emit_ph(t + 1, inp[1])

                for m in range(MH):
                    nc.vector.tensor_scalar(
                        q1[:, m, :], p4[m][:], sob1_s[:, m : m + 1], 0.0, add, mx
                    )

                # L2 for both heads, drains interleaved so the following
                # matmul stream hides each drain
                a2 = acts.tile([128, 2, NT], sdt, tag="a2", name=f"a2_{t}")
                q2 = acts.tile([128, 2, NT], sdt, tag="q2", name=f"q2_{t}")
                p2 = [psum.tile([128, NT], f32, tag="ps", name=f"p2_{t}_{i}") for i in range(MH)]
                p5 = [psum.tile([128, NT], f32, tag="ps", name=f"p5_{t}_{i}") for i in range(MH)]
                for m in range(MH):
                    for k in range(KH):
                        nc.tensor.matmul(
                            p2[m][:],
                            lhsT=w2(k, m),
                            rhs=a1[:, k, :],
                            start=(k == 0),
                            stop=(k == KH - 1),
                        )
                    nc.vector.tensor_scalar(
                        a2[:, m, :], p2[m][:], bht[:, 2 + m : 3 + m], 0.0, add, mx
                    )
                for m in range(MH):
                    for k in range(KH):
                        nc.tensor.matmul(
                            p5[m][:],
                            lhsT=soW2_s[:, k, ts(m, 128)],
                            rhs=q1[:, k, :],
                            start=(k == 0),
                            stop=(k == KH - 1),
                        )
                    nc.vector.tensor_scalar(
                        q2[:, m, :], p5[m][:], sob2_s[:, m : m + 1], 0.0, add, mx
                    )

                # L3: branch logits -> sigmoid; speed head -> identity
                p3 = psum.tile([3, NT], f32, tag="ps", name=f"p3_{t}")
                for k in range(KH):
                    nc.tensor.matmul(
                        p3[:],
                        lhsT=w3(k),
                        rhs=a2[:, k, :],
                        start=(k == 0),
                        stop=(k == KH - 1),
                    )
                cout = outs.tile([3, NT], f32, tag="c", name=f"cout{t}")
                nc.scalar.activation(cout[:], p3[:], Sig, bias=bht[0:3, 4:5])
                nc.sync.dma_start(out=ctrl[:, bsl], in_=cout[:])

                p6 = psum.tile([1, NT], f32, tag="ps", name=f"p6_{t}")
                for k in range(KH):
                    nc.tensor.matmul(
                        p6[:],
                        lhsT=soW3_s[:, k, :],
                        rhs=q2[:, k, :],
                        start=(k == 0),
                        stop=(k == KH - 1),
                    )
                sout = outs.tile([1, NT], f32, tag="sv", name=f"sout{t}")
                nc.scalar.activation(sout[:], p6[:], Ident, bias=sob3_s[0:1, 0:1])
                nc.sync.dma_start(out=spd[0:1, bsl], in_=sout[:])

    nc.finalize()
    return nc


def _get_nc():
    if "nc" not in _cached:
        _cached["nc"] = _build()
    return _cached["nc"]


def _route(command):
    """Bucket samples by command into single-command tiles of NT.

    Returns (slots, tile_cmd, nreal): slots[NCORES*CAP] maps device slot ->
    original sample index (bucket tails padded with a repeated in-bucket
    sample; trailing dummy tiles use sample 0), tile_cmd[NCORES*TILES] gives
    each tile's branch id, and slots[nreal:] are dummy-tile slots whose
    outputs must not be scattered back.
    """
    cmd = np.clip(np.asarray(command).astype(np.int64) - 1, 0, NB - 1)
    order = np.argsort(cmd, kind="stable")
    counts = np.bincount(cmd, minlength=NB)
    pieces = []
    tile_cmd = []
    pos = 0
    for c in range(NB):
        n = int(counts[c])
        if n == 0:
            continue
        idxs = order[pos : pos + n]
        pos += n
        ntile = -(-n // NT)
        pad = ntile * NT - n
        if pad:
            idxs = np.concatenate([idxs, np.full(pad, idxs[0], np.int64)])
        pieces.append(idxs)
        tile_cmd.extend([c] * ntile)
    nreal = len(tile_cmd) * NT
    ndum = NCORES * TILES - len(tile_cmd)
    assert ndum >= 0, "tile budget exceeded"
    if ndum:
        pieces.append(np.zeros(ndum * NT, np.int64))
        tile_cmd.extend([0] * ndum)
    return np.concatenate(pieces), np.asarray(tile_cmd), nreal


def _prep_in_maps(inputs, slots, tile_cmd):
    import ml_dtypes

    sdt_np = ml_dtypes.bfloat16 if COMPUTE_MODE == "bf16" else np.float32

    def s(x):  # storage-dtype cast
        return np.ascontiguousarray(np.asarray(x, np.float32).astype(sdt_np))

    def f(x):  # always-f32 (biases)
        return np.ascontiguousarray(np.asarray(x, np.float32))

    emb = np.asarray(inputs["embedding"], np.float32)
    speed = np.asarray(inputs["speed"], np.float32)

    gx = emb[slots]                                  # [NCORES*CAP, E]
    gs = speed[slots, 0]                             # [NCORES*CAP]

    # packed per-command weight/bias images (gathered per tile below)
    bW1 = np.asarray(inputs["bW1"], np.float32)
    bb1 = np.asarray(inputs["bb1"], np.float32)
    bW2 = np.asarray(inputs["bW2"], np.float32)
    bb2 = np.asarray(inputs["bb2"], np.float32)
    bW3 = np.asarray(inputs["bW3"], np.float32)
    bb3 = np.asarray(inputs["bb3"], np.float32)
    pw = np.empty((NB, 128, WCOLS), np.float32)
    pb = np.zeros((NB, 128, 5), np.float32)
    for c in range(NB):
        w1 = bW1[c].reshape(KD, 128, H).transpose(1, 0, 2).reshape(128, KD * H)
        w2 = bW2[c].reshape(KH, 128, H).transpose(1, 0, 2).reshape(128, KH * H)
        w3 = bW3[c].reshape(KH, 128, 3).transpose(1, 0, 2).reshape(128, KH * 3)
        pw[c] = np.concatenate([w1, w2, w3], axis=1)
        pb[c, :, 0:2] = bb1[c].reshape(2, 128).T
        pb[c, :, 2:4] = bb2[c].reshape(2, 128).T
        pb[c, 0:3, 4] = bb3[c]
    pw = pw.astype(sdt_np)

    shared = {
        "siW1": s(inputs["si_W1"].reshape(1, H)),
        "sib1": f(inputs["si_b1"]),
        "siW2": s(inputs["si_W2"]),
        "sib2": f(inputs["si_b2"]),
        "soW1": s(inputs["so_W1"]),
        "sob1": f(inputs["so_b1"]),
        "soW2": s(inputs["so_W2"]),
        "sob2": f(inputs["so_b2"]),
        "soW3": s(inputs["so_W3"]),
        "sob3": f(inputs["so_b3"].reshape(1, 1)),
    }

    in_maps = []
    for c in range(NCORES):
        sl = slice(c * CAP, (c + 1) * CAP)
        tc_ = tile_cmd[c * TILES : (c + 1) * TILES]
        # [tile, partition(=dim within K-tile), K-tile, sample]
        xc = gx[sl].reshape(TILES, NT, KE, 128).transpose(0, 3, 2, 1)
        in_map = {
            "xh": s(xc),
            "sp": s(gs[sl].reshape(1, CAP)),
            "wh": np.ascontiguousarray(pw[tc_]),
            "bh": np.ascontiguousarray(pb[tc_]),
        }
        in_map.update(shared)
        in_maps.append(in_map)
    return in_maps


def _run(inputs, trace=False):
    from concourse.bass_utils import run_bass_kernel_spmd

    command = np.asarray(inputs["command"])
    assert command.shape == (B,), command.shape
    slots, tile_cmd, nreal = _route(command)
    in_maps = _prep_in_maps(inputs, slots, tile_cmd)

    nc = _get_nc()
    res = run_bass_kernel_spmd(nc, in_maps, list(range(NCORES)), trace=trace)

    control = np.zeros((B, 3), np.float32)
    speed_pred = np.zeros((B, 1), np.float32)
    for c in range(NCORES):
        lo = c * CAP
        nvalid = min(max(nreal - lo, 0), CAP)
        if nvalid == 0:
            continue
        sl = slots[lo : lo + nvalid]
        control[sl] = res.results[c]["ctrl"][:, :nvalid].T
        speed_pred[sl, 0] = res.results[c]["spd"][0, :nvalid]

    # Out-of-range commands select no branch in the reference -> zeros.
    bad = (command < 1) | (command > NB)
    if bad.any():
        control[bad] = 0.0
    return control, speed_pred, res


def kernel(**inputs):
    control, speed_pred, _ = _run(inputs, trace=False)
    return control, speed_pred


# revision 14
# speedup vs baseline: 1.3540x; 1.0457x over previous
"""Trainium2 Bass kernel for nn_CILRSVAEModel (CILRS-style MoE branch routing).

Model (per sample):
  s      = relu(speed @ si_W1 + si_b1) @ si_W2 + si_b2            # [SL]
  emb    = concat(embedding, s)                                   # [D=640]
  branch = command-selected MLP: sigmoid(relu(relu(emb@W1+b1)@W2+b2)@W3+b3)
  speed_pred = relu(relu(emb@so_W1+so_b1)@so_W2+so_b2)@so_W3+so_b3

Strategy: pure data parallel over batch, plus host-side MoE routing — samples
are bucketed by `command` into single-command tiles of 512, so the device
computes only the selected branch per sample (6x fewer branch FLOPs than the
dense reference). Each tile's branch weights are gathered host-side into a
per-tile packed weight image, keeping the device program fully static. All
device-facing arrays are packed host-side into [128-partition, contiguous]
layouts so every DMA is 128 descriptors of multi-KB contiguous runs, and the
embedding is pre-transposed so activations live in [feature, batch] layout
end-to-end (no on-device transposes). Matmuls run as lhsT=[K,M] weights x
rhs=[K,N] activations with N=512 batch columns, in fp32r (full PE rate).
PSUM->SBUF bias+activation drains are split between the Scalar (ACT) and
Vector (DVE) engines; relu/sigmoid/identity share one ACT table set so there
are no table reloads.
"""

import os

# The device program runs through jax's axon/PJRT backend; make sure a
# harness-set JAX_PLATFORMS=cpu doesn't hide the neuron devices.
if os.environ.get("JAX_PLATFORMS", None) in ("cpu",):
    os.environ.pop("JAX_PLATFORMS")

import sys

import numpy as np

for _p in ("/opt/trn_rl_repo",):
    if _p not in sys.path and os.path.isdir(_p):
        sys.path.insert(0, _p)

B = 65536
E = 512
SL = 128
H = 256
NB = 6
D = E + SL
NCORES = 8
NT = 512                    # samples per tile (= fp32 matmul moving-dim max)
TILES = 17                  # tiles per core; 8*17=136 >= worst case 134
CAP = NT * TILES            # samples per core incl. padding

KD = D // 128               # 5 K-tiles of the branch input dim
KE = E // 128               # 4 of them come straight from the embedding
KH = H // 128               # 2 K-tiles of the hidden dim
MH = H // 128               # 2 M-tiles of the hidden dim

# packed per-tile weight image: per partition p (=row within a K-tile):
#   [ W1 (KD*H) | W2 (KH*H) | W3 (KH*3) ]
W2OFF = KD * H              # 1280
W3OFF = W2OFF + KH * H      # 1792
WCOLS = W3OFF + KH * 3      # 1798

# packed shared-weight image column offsets
SOW2OFF = KD * H            # 1280
SIW2OFF = SOW2OFF + KH * H  # 1792
SOW3OFF = SIW2OFF + KH * SL  # 2048
FWCOLS = SOW3OFF + KH       # 2050

# "f32r": fp32 storage, fp32r (tf32-like full-rate) matmuls.
# "bf16": bf16 storage/matmuls (half the DMA), fp32 accumulate.
COMPUTE_MODE = os.environ.get("KERNEL_DT", "f32r")

_cached = {}


def _build():
    """Build + finalize the SPMD Bacc program (identical on all 8 cores)."""
    from concourse import bacc, mybir
    import concourse.tile as tile
    from concourse.bass import ts

    f32 = mybir.dt.float32
    bf16 = mybir.dt.bfloat16
    # storage dtype for everything the matmul touches. fp32r runs the PE at
    # full rate (vs 1/4 for fp32) with tf32-like precision; walrus requires
    # the whole producer chain (DMA, DVE) to carry the fp32r dtype, so the
    # tensors are declared fp32r end-to-end (numpy side is plain float32).
    sdt = bf16 if COMPUTE_MODE == "bf16" else mybir.dt.float32r

    add = mybir.AluOpType.add
    mx = mybir.AluOpType.max
    Relu = mybir.ActivationFunctionType.Relu
    Ident = mybir.ActivationFunctionType.Identity
    Sig = mybir.ActivationFunctionType.Sigmoid

    nc = bacc.Bacc(None, target_bir_lowering=False)

    # --- I/O ---------------------------------------------------------------
    xh = nc.declare_dram_parameter("xh", [TILES, 128, KE, NT], sdt, isOutput=False)
    sp = nc.declare_dram_parameter("sp", [1, CAP], sdt, isOutput=False)
    wh = nc.declare_dram_parameter("wh", [TILES, 128, WCOLS], sdt, isOutput=False)
    bh = nc.declare_dram_parameter("bh", [TILES, 128, 5], f32, isOutput=False)
    # packed shared weights: [ soW1 (KD*H) | soW2 (KH*H) | siW2 (KH*SL) | soW3 (KH) ]
    fw = nc.declare_dram_parameter("fw", [128, FWCOLS], sdt, isOutput=False)
    # packed shared biases: [ sib1 (2) | sob1 (2) | sob2 (2) | sib2 (1) ]
    fb = nc.declare_dram_parameter("fb", [128, 7], f32, isOutput=False)
    siW1 = nc.declare_dram_parameter("siW1", [1, H], sdt, isOutput=False)
    sob3 = nc.declare_dram_parameter("sob3", [1, 1], f32, isOutput=False)
    ctrl = nc.declare_dram_parameter("ctrl", [3, CAP], f32, isOutput=True)
    spd = nc.declare_dram_parameter("spd", [1, CAP], f32, isOutput=True)

    with tile.TileContext(nc) as tc:
        with (
            tc.tile_pool(name="fixed", bufs=1) as fixed,
            tc.tile_pool(name="wts", bufs=3) as wts,
            tc.tile_pool(name="xin", bufs=3) as xin,
            tc.tile_pool(name="acts", bufs=2) as acts,
            tc.tile_pool(name="outs", bufs=3) as outs,
            tc.tile_pool(name="psum", bufs=8, space="PSUM") as psum,
        ):
            def load_inputs(t):
                """Issue the input DMAs for tile t (one tile of prefetch)."""
                xt = xin.tile([128, KE, NT], sdt, tag="xt", name=f"xt{t}")
                nc.sync.dma_start(out=xt[:], in_=xh[t])
                spt = xin.tile([1, NT], sdt, tag="spt", name=f"spt{t}")
                nc.sync.dma_start(out=spt[:], in_=sp[0:1, ts(t, NT)])
                wht = wts.tile([128, WCOLS], sdt, tag="w", name=f"wht{t}")
                nc.sync.dma_start(out=wht[:], in_=wh[t])
                bht = wts.tile([128, 5], f32, tag="b", name=f"bht{t}")
                nc.sync.dma_start(out=bht[:], in_=bh[t])
                return xt, spt, wht, bht

            def emit_ph(t, spt):
                """speed_in L1 matmuls for tile t (K=1 outer products)."""
                ph = [
                    psum.tile([128, NT], f32, tag="ps", name=f"ph{t}_{i}")
                    for i in range(2)
                ]
                for m in range(2):
                    nc.tensor.matmul(
                        ph[m][:],
                        lhsT=siW1_s[0:1, ts(m, 128)],
                        rhs=spt[:],
                        start=True,
                        stop=True,
                    )
                return ph

            # tile-0 inputs first so compute starts as soon as they land;
            # the packed shared-weight images load in parallel on other
            # queues and are first needed a few matmul-groups in.
            inp = load_inputs(0)
            siW1_s = fixed.tile([1, H], sdt)
            nc.sync.dma_start(out=siW1_s[:], in_=siW1[:])
            fb_s = fixed.tile([128, 7], f32)
            nc.sync.dma_start(out=fb_s[:], in_=fb[:])
            fw_s = fixed.tile([128, FWCOLS], sdt)
            nc.sync.dma_start(out=fw_s[:], in_=fw[:])
            sob3_s = fixed.tile([1, 1], f32)
            nc.sync.dma_start(out=sob3_s[:], in_=sob3[:])

            def soW1_a(k, m):
                return fw_s[:, k * H + m * 128 : k * H + (m + 1) * 128]

            def soW2_a(k, m):
                return fw_s[:, SOW2OFF + k * H + m * 128 : SOW2OFF + k * H + (m + 1) * 128]

            def siW2_a(k):
                return fw_s[:, SIW2OFF + k * SL : SIW2OFF + (k + 1) * SL]

            def soW3_a(k):
                return fw_s[:, SOW3OFF + k : SOW3OFF + k + 1]

            def sib1_a(m):
                return fb_s[:, m : m + 1]

            def sob1_a(m):
                return fb_s[:, 2 + m : 3 + m]

            def sob2_a(m):
                return fb_s[:, 4 + m : 5 + m]

            sib2_a = fb_s[:, 6:7]

            ph = emit_ph(0, inp[1])

            for t in range(TILES):
                bsl = ts(t, NT)
                xt, spt, wht, bht = inp

                def w1(k, m):
                    return wht[:, k * H + m * 128 : k * H + (m + 1) * 128]

                def w2(k, m):
                    return wht[:, W2OFF + k * H + m * 128 : W2OFF + k * H + (m + 1) * 128]

                def w3(k):
                    return wht[:, W3OFF + k * 3 : W3OFF + (k + 1) * 3]

                # prefetch next tile's inputs a full tile ahead
                if t + 1 < TILES:
                    inp = load_inputs(t + 1)

                # drain speed_in L1 (ACT) while the PE starts on this tile's
                # embedding matmuls
                hst = acts.tile([128, 2, NT], sdt, tag="h", name=f"hst{t}")
                for m in range(2):
                    nc.scalar.activation(
                        hst[:, m, :], ph[m][:], Relu, bias=sib1_a(m)
                    )

                # branch L1 over the embedding K-tiles (k=0..3); the 5th
                # K-tile (s) joins once computed.
                p1 = [psum.tile([128, NT], f32, tag="ps", name=f"p1_{t}_{i}") for i in range(MH)]
                p4 = [psum.tile([128, NT], f32, tag="ps", name=f"p4_{t}_{i}") for i in range(MH)]
                for m in range(MH):
                    for k in range(KE):
                        nc.tensor.matmul(
                            p1[m][:],
                            lhsT=w1(k, m),
                            rhs=xt[:, k, :],
                            start=(k == 0),
                            stop=False,
                        )

                # speed_in L2 (no relu): s_T[l,b]
                ps = psum.tile([128, NT], f32, tag="ps", name=f"ps{t}")
                for k in range(KH):
                    nc.tensor.matmul(
                        ps[:],
                        lhsT=siW2_a(k),
                        rhs=hst[:, k, :],
                        start=(k == 0),
                        stop=(k == KH - 1),
                    )
                sst = acts.tile([128, NT], sdt, tag="s", name=f"sst{t}")
                nc.scalar.activation(sst[:], ps[:], Ident, bias=sib2_a)

                # speed-head L1 embedding K-tiles keep the PE busy while ACT
                # produces s
                for m in range(MH):
                    for k in range(KE):
                        nc.tensor.matmul(
                            p4[m][:],
                            lhsT=soW1_a(k, m),
                            rhs=xt[:, k, :],
                            start=(k == 0),
                            stop=False,
                        )

                # close the L1 groups with the s K-tile; drain each group the
                # moment its last matmul lands
                a1 = acts.tile([128, 2, NT], sdt, tag="a1", name=f"a1_{t}")
                q1 = acts.tile([128, 2, NT], sdt, tag="q1", name=f"q1_{t}")
                for m in range(MH):
                    nc.tensor.matmul(
                        p1[m][:], lhsT=w1(KE, m), rhs=sst[:], start=False, stop=True
                    )
                for m in range(MH):
                    nc.vector.tensor_scalar(
                        a1[:, m, :], p1[m][:], bht[:, m : m + 1], 0.0, add, mx
                    )
                for m in range(MH):
                    nc.tensor.matmul(
                        p4[m][:],
                        lhsT=soW1_a(KE, m),
                        rhs=sst[:],
                        start=False,
                        stop=True,
                    )

                # next tile's speed_in L1 fills the PE while the DVE drains a1
                if t + 1 < TILES:
                    ph = emit_ph(t + 1, inp[1])

                for m in range(MH):
                    nc.vector.tensor_scalar(
                        q1[:, m, :], p4[m][:], sob1_a(m), 0.0, add, mx
                    )

                # L2 for both heads, drains interleaved so the following
                # matmul stream hides each drain
                a2 = acts.tile([128, 2, NT], sdt, tag="a2", name=f"a2_{t}")
                q2 = acts.tile([128, 2, NT], sdt, tag="q2", name=f"q2_{t}")
                p2 = [psum.tile([128, NT], f32, tag="ps", name=f"p2_{t}_{i}") for i in range(MH)]
                p5 = [psum.tile([128, NT], f32, tag="ps", name=f"p5_{t}_{i}") for i in range(MH)]
                for m in range(MH):
                    for k in range(KH):
                        nc.tensor.matmul(
                            p2[m][:],
                            lhsT=w2(k, m),
                            rhs=a1[:, k, :],
                            start=(k == 0),
                            stop=(k == KH - 1),
                        )
                    nc.vector.tensor_scalar(
                        a2[:, m, :], p2[m][:], bht[:, 2 + m : 3 + m], 0.0, add, mx
                    )
                for m in range(MH):
                    for k in range(KH):
                        nc.tensor.matmul(
                            p5[m][:],
                            lhsT=soW2_a(k, m),
                            rhs=q1[:, k, :],
                            start=(k == 0),
                            stop=(k == KH - 1),
                        )
                    nc.vector.tensor_scalar(
                        q2[:, m, :], p5[m][:], sob2_a(m), 0.0, add, mx
                    )

                # L3: branch logits -> sigmoid; speed head -> identity
                p3 = psum.tile([3, NT], f32, tag="ps", name=f"p3_{t}")
                for k in range(KH):
                    nc.tensor.matmul(
                        p3[:],
                        lhsT=w3(k),
                        rhs=a2[:, k, :],
                        start=(k == 0),
                        stop=(k == KH - 1),
                    )
                cout = outs.tile([3, NT], f32, tag="c", name=f"cout{t}")
                nc.scalar.activation(cout[:], p3[:], Sig, bias=bht[0:3, 4:5])
                nc.sync.dma_start(out=ctrl[:, bsl], in_=cout[:])

                p6 = psum.tile([1, NT], f32, tag="ps", name=f"p6_{t}")
                for k in range(KH):
                    nc.tensor.matmul(
                        p6[:],
                        lhsT=soW3_a(k),
                        rhs=q2[:, k, :],
                        start=(k == 0),
                        stop=(k == KH - 1),
                    )
                sout = outs.tile([1, NT], f32, tag="sv", name=f"sout{t}")
                nc.scalar.activation(sout[:], p6[:], Ident, bias=sob3_s[0:1, 0:1])
                nc.sync.dma_start(out=spd[0:1, bsl], in_=sout[:])

    nc.finalize()
    return nc


def _get_nc():
    if "nc" not in _cached:
        _cached["nc"] = _build()
    return _cached["nc"]


def _route(command):
    """Bucket samples by command into single-command tiles of NT.

    Returns (slots, tile_cmd, nreal): slots[NCORES*CAP] maps device slot ->
    original sample index (bucket tails padded with a repeated in-bucket
    sample; trailing dummy tiles use sample 0), tile_cmd[NCORES*TILES] gives
    each tile's branch id, and slots[nreal:] are dummy-tile slots whose
    outputs must not be scattered back.
    """
    cmd = np.clip(np.asarray(command).astype(np.int64) - 1, 0, NB - 1)
    order = np.argsort(cmd, kind="stable")
    counts = np.bincount(cmd, minlength=NB)
    pieces = []
    tile_cmd = []
    pos = 0
    for c in range(NB):
        n = int(counts[c])
        if n == 0:
            continue
        idxs = order[pos : pos + n]
        pos += n
        ntile = -(-n // NT)
        pad = ntile * NT - n
        if pad:
            idxs = np.concatenate([idxs, np.full(pad, idxs[0], np.int64)])
        pieces.append(idxs)
        tile_cmd.extend([c] * ntile)
    nreal = len(tile_cmd) * NT
    ndum = NCORES * TILES - len(tile_cmd)
    assert ndum >= 0, "tile budget exceeded"
    if ndum:
        pieces.append(np.zeros(ndum * NT, np.int64))
        tile_cmd.extend([0] * ndum)
    return np.concatenate(pieces), np.asarray(tile_cmd), nreal


def _prep_in_maps(inputs, slots, tile_cmd):
    import ml_dtypes

    sdt_np = ml_dtypes.bfloat16 if COMPUTE_MODE == "bf16" else np.float32

    def s(x):  # storage-dtype cast
        return np.ascontiguousarray(np.asarray(x, np.float32).astype(sdt_np))

    def f(x):  # always-f32 (biases)
        return np.ascontiguousarray(np.asarray(x, np.float32))

    emb = np.asarray(inputs["embedding"], np.float32)
    speed = np.asarray(inputs["speed"], np.float32)

    gx = emb[slots]                                  # [NCORES*CAP, E]
    gs = speed[slots, 0]                             # [NCORES*CAP]

    # packed per-command weight/bias images (gathered per tile below)
    bW1 = np.asarray(inputs["bW1"], np.float32)
    bb1 = np.asarray(inputs["bb1"], np.float32)
    bW2 = np.asarray(inputs["bW2"], np.float32)
    bb2 = np.asarray(inputs["bb2"], np.float32)
    bW3 = np.asarray(inputs["bW3"], np.float32)
    bb3 = np.asarray(inputs["bb3"], np.float32)
    pw = np.empty((NB, 128, WCOLS), np.float32)
    pb = np.zeros((NB, 128, 5), np.float32)
    for c in range(NB):
        w1 = bW1[c].reshape(KD, 128, H).transpose(1, 0, 2).reshape(128, KD * H)
        w2 = bW2[c].reshape(KH, 128, H).transpose(1, 0, 2).reshape(128, KH * H)
        w3 = bW3[c].reshape(KH, 128, 3).transpose(1, 0, 2).reshape(128, KH * 3)
        pw[c] = np.concatenate([w1, w2, w3], axis=1)
        pb[c, :, 0:2] = bb1[c].reshape(2, 128).T
        pb[c, :, 2:4] = bb2[c].reshape(2, 128).T
        pb[c, 0:3, 4] = bb3[c]
    pw = pw.astype(sdt_np)

    # packed shared-weight image [128, FWCOLS] and bias image [128, 7]
    soW1p = (
        np.asarray(inputs["so_W1"], np.float32)
        .reshape(KD, 128, H).transpose(1, 0, 2).reshape(128, KD * H)
    )
    soW2p = (
        np.asarray(inputs["so_W2"], np.float32)
        .reshape(KH, 128, H).transpose(1, 0, 2).reshape(128, KH * H)
    )
    siW2p = (
        np.asarray(inputs["si_W2"], np.float32)
        .reshape(KH, 128, SL).transpose(1, 0, 2).reshape(128, KH * SL)
    )
    soW3p = (
        np.asarray(inputs["so_W3"], np.float32)
        .reshape(KH, 128, 1).transpose(1, 0, 2).reshape(128, KH)
    )
    fwp = np.concatenate([soW1p, soW2p, siW2p, soW3p], axis=1)
    fbp = np.zeros((128, 7), np.float32)
    fbp[:, 0:2] = np.asarray(inputs["si_b1"], np.float32).reshape(2, 128).T
    fbp[:, 2:4] = np.asarray(inputs["so_b1"], np.float32).reshape(2, 128).T
    fbp[:, 4:6] = np.asarray(inputs["so_b2"], np.float32).reshape(2, 128).T
    fbp[:, 6] = np.asarray(inputs["si_b2"], np.float32).reshape(1, 128)[0]
    shared = {
        "fw": s(fwp),
        "fb": f(fbp),
        "siW1": s(inputs["si_W1"].reshape(1, H)),
        "sob3": f(inputs["so_b3"].reshape(1, 1)),
    }

    in_maps = []
    for c in range(NCORES):
        sl = slice(c * CAP, (c + 1) * CAP)
        tc_ = tile_cmd[c * TILES : (c + 1) * TILES]
        # [tile, partition(=dim within K-tile), K-tile, sample]
        xc = gx[sl].reshape(TILES, NT, KE, 128).transpose(0, 3, 2, 1)
        in_map = {
            "xh": s(xc),
            "sp": s(gs[sl].reshape(1, CAP)),
            "wh": np.ascontiguousarray(pw[tc_]),
            "bh": np.ascontiguousarray(pb[tc_]),
        }
        in_map.update(shared)
        in_maps.append(in_map)
    return in_maps


def _run(inputs, trace=False):
    from concourse.bass_utils import run_bass_kernel_spmd

    command = np.asarray(inputs["command"])
    assert command.shape == (B,), command.shape
    slots, tile_cmd, nreal = _route(command)
    in_maps = _prep_in_maps(inputs, slots, tile_cmd)

    nc = _get_nc()
    res = run_bass_kernel_spmd(nc, in_maps, list(range(NCORES)), trace=trace)

    control = np.zeros((B, 3), np.float32)
    speed_pred = np.zeros((B, 1), np.float32)
    for c in range(NCORES):
        lo = c * CAP
        nvalid = min(max(nreal - lo, 0), CAP)
        if nvalid == 0:
            continue
        sl = slots[lo : lo + nvalid]
        control[sl] = res.results[c]["ctrl"][:, :nvalid].T
        speed_pred[sl, 0] = res.results[c]["spd"][0, :nvalid]

    # Out-of-range commands select no branch in the reference -> zeros.
    bad = (command < 1) | (command > NB)
    if bad.any():
        control[bad] = 0.0
    return control, speed_pred, res


def kernel(**inputs):
    control, speed_pred, _ = _run(inputs, trace=False)
    return control, speed_pred


# revision 17
# speedup vs baseline: 1.3795x; 1.0188x over previous
"""Trainium2 Bass kernel for nn_CILRSVAEModel (CILRS-style MoE branch routing).

Model (per sample):
  s      = relu(speed @ si_W1 + si_b1) @ si_W2 + si_b2            # [SL]
  emb    = concat(embedding, s)                                   # [D=640]
  branch = command-selected MLP: sigmoid(relu(relu(emb@W1+b1)@W2+b2)@W3+b3)
  speed_pred = relu(relu(emb@so_W1+so_b1)@so_W2+so_b2)@so_W3+so_b3

Strategy: pure data parallel over batch, plus host-side MoE routing — samples
are bucketed by `command` into single-command tiles of 512, so the device
computes only the selected branch per sample (6x fewer branch FLOPs than the
dense reference). Each tile's branch weights are gathered host-side into a
per-tile packed weight image, keeping the device program fully static. All
device-facing arrays are packed host-side into [128-partition, contiguous]
layouts so every DMA is 128 descriptors of multi-KB contiguous runs, and the
embedding is pre-transposed so activations live in [feature, batch] layout
end-to-end (no on-device transposes). Matmuls run as lhsT=[K,M] weights x
rhs=[K,N] activations with N=512 batch columns, in fp32r (full PE rate).
PSUM->SBUF bias+activation drains are split between the Scalar (ACT) and
Vector (DVE) engines; relu/sigmoid/identity share one ACT table set so there
are no table reloads.
"""

import os

# The device program runs through jax's axon/PJRT backend; make sure a
# harness-set JAX_PLATFORMS=cpu doesn't hide the neuron devices.
if os.environ.get("JAX_PLATFORMS", None) in ("cpu",):
    os.environ.pop("JAX_PLATFORMS")

import sys

import numpy as np

for _p in ("/opt/trn_rl_repo",):
    if _p not in sys.path and os.path.isdir(_p):
        sys.path.insert(0, _p)

B = 65536
E = 512
SL = 128
H = 256
NB = 6
D = E + SL
NCORES = 8
NT = 512                    # samples per tile (= fp32 matmul moving-dim max)
TILES = 17                  # tiles per core; 8*17=136 >= worst case 134
CAP = NT * TILES            # samples per core incl. padding

KD = D // 128               # 5 K-tiles of the branch input dim
KE = E // 128               # 4 of them come straight from the embedding
KH = H // 128               # 2 K-tiles of the hidden dim
MH = H // 128               # 2 M-tiles of the hidden dim

# packed per-tile weight image: per partition p (=row within a K-tile):
#   [ W1 (KD*H) | W2 (KH*H) | W3 (KH*3) ]
W2OFF = KD * H              # 1280
W3OFF = W2OFF + KH * H      # 1792
WCOLS = W3OFF + KH * 3      # 1798

# packed shared-weight image column offsets
SOW2OFF = KD * H            # 1280
SIW2OFF = SOW2OFF + KH * H  # 1792
SOW3OFF = SIW2OFF + KH * SL  # 2048
FWCOLS = SOW3OFF + KH       # 2050

# "f32r": fp32 storage, fp32r (tf32-like full-rate) matmuls.
# "bf16": bf16 storage/matmuls (half the DMA), fp32 accumulate.
COMPUTE_MODE = os.environ.get("KERNEL_DT", "f32r")

_cached = {}


def _build():
    """Build + finalize the SPMD Bacc program (identical on all 8 cores)."""
    from concourse import bacc, mybir
    import concourse.tile as tile
    from concourse.bass import ts

    f32 = mybir.dt.float32
    bf16 = mybir.dt.bfloat16
    # storage dtype for everything the matmul touches. fp32r runs the PE at
    # full rate (vs 1/4 for fp32) with tf32-like precision; walrus requires
    # the whole producer chain (DMA, DVE) to carry the fp32r dtype, so the
    # tensors are declared fp32r end-to-end (numpy side is plain float32).
    sdt = bf16 if COMPUTE_MODE == "bf16" else mybir.dt.float32r

    add = mybir.AluOpType.add
    mx = mybir.AluOpType.max
    Relu = mybir.ActivationFunctionType.Relu
    Ident = mybir.ActivationFunctionType.Identity
    Sig = mybir.ActivationFunctionType.Sigmoid

    nc = bacc.Bacc(None, target_bir_lowering=False)

    # --- I/O ---------------------------------------------------------------
    xh = nc.declare_dram_parameter("xh", [TILES, 128, KE, NT], sdt, isOutput=False)
    sp = nc.declare_dram_parameter("sp", [1, CAP], sdt, isOutput=False)
    wh = nc.declare_dram_parameter("wh", [TILES, 128, WCOLS], sdt, isOutput=False)
    bh = nc.declare_dram_parameter("bh", [TILES, 128, 5], f32, isOutput=False)
    # packed shared weights: [ soW1 (KD*H) | soW2 (KH*H) | siW2 (KH*SL) | soW3 (KH) ]
    fw = nc.declare_dram_parameter("fw", [128, FWCOLS], sdt, isOutput=False)
    # packed shared biases: [ sib1 (2) | sob1 (2) | sob2 (2) | sib2 (1) ]
    fb = nc.declare_dram_parameter("fb", [128, 7], f32, isOutput=False)
    siW1 = nc.declare_dram_parameter("siW1", [1, H], sdt, isOutput=False)
    sob3 = nc.declare_dram_parameter("sob3", [1, 1], f32, isOutput=False)
    ctrl = nc.declare_dram_parameter("ctrl", [3, CAP], f32, isOutput=True)
    spd = nc.declare_dram_parameter("spd", [1, CAP], f32, isOutput=True)

    with tile.TileContext(nc) as tc:
        with (
            tc.tile_pool(name="fixed", bufs=1) as fixed,
            tc.tile_pool(name="wts", bufs=3) as wts,
            tc.tile_pool(name="xin", bufs=3) as xin,
            tc.tile_pool(name="acts", bufs=2) as acts,
            tc.tile_pool(name="outs", bufs=3) as outs,
            tc.tile_pool(name="psum", bufs=8, space="PSUM") as psum,
        ):
            def load_inputs(t):
                """Issue the input DMAs for tile t (one tile of prefetch)."""
                xt = xin.tile([128, KE, NT], sdt, tag="xt", name=f"xt{t}")
                nc.sync.dma_start(out=xt[:], in_=xh[t])
                spt = xin.tile([1, NT], sdt, tag="spt", name=f"spt{t}")
                nc.sync.dma_start(out=spt[:], in_=sp[0:1, ts(t, NT)])
                wht = wts.tile([128, WCOLS], sdt, tag="w", name=f"wht{t}")
                nc.sync.dma_start(out=wht[:], in_=wh[t])
                bht = wts.tile([128, 5], f32, tag="b", name=f"bht{t}")
                nc.sync.dma_start(out=bht[:], in_=bh[t])
                return xt, spt, wht, bht

            def emit_ph(t, spt):
                """speed_in L1 matmuls for tile t (K=1 outer products)."""
                ph = [
                    psum.tile([128, NT], f32, tag="ps", name=f"ph{t}_{i}")
                    for i in range(2)
                ]
                for m in range(2):
                    nc.tensor.matmul(
                        ph[m][:],
                        lhsT=siW1_s[0:1, ts(m, 128)],
                        rhs=spt[:],
                        start=True,
                        stop=True,
                    )
                return ph

            # tile-0 inputs first so compute starts as soon as they land;
            # the packed shared-weight images load in parallel on other
            # queues and are first needed a few matmul-groups in.
            inp = load_inputs(0)
            siW1_s = fixed.tile([1, H], sdt)
            nc.sync.dma_start(out=siW1_s[:], in_=siW1[:])
            fb_s = fixed.tile([128, 7], f32)
            nc.sync.dma_start(out=fb_s[:], in_=fb[:])
            sob3_s = fixed.tile([1, 1], f32)
            nc.sync.dma_start(out=sob3_s[:], in_=sob3[:])
            # the big shared-weight image is first needed a few matmul groups
            # into tile 0 — keep it off the critical xt0/wht0 transfer path
            fw_s = fixed.tile([128, FWCOLS], sdt)

            def soW1_a(k, m):
                return fw_s[:, k * H + m * 128 : k * H + (m + 1) * 128]

            def soW2_a(k, m):
                return fw_s[:, SOW2OFF + k * H + m * 128 : SOW2OFF + k * H + (m + 1) * 128]

            def siW2_a(k):
                return fw_s[:, SIW2OFF + k * SL : SIW2OFF + (k + 1) * SL]

            def soW3_a(k):
                return fw_s[:, SOW3OFF + k : SOW3OFF + k + 1]

            def sib1_a(m):
                return fb_s[:, m : m + 1]

            def sob1_a(m):
                return fb_s[:, 2 + m : 3 + m]

            def sob2_a(m):
                return fb_s[:, 4 + m : 5 + m]

            sib2_a = fb_s[:, 6:7]

            ph = emit_ph(0, inp[1])
            nc.sync.dma_start(out=fw_s[:], in_=fw[:])

            for t in range(TILES):
                bsl = ts(t, NT)
                xt, spt, wht, bht = inp

                def w1(k, m):
                    return wht[:, k * H + m * 128 : k * H + (m + 1) * 128]

                def w2(k, m):
                    return wht[:, W2OFF + k * H + m * 128 : W2OFF + k * H + (m + 1) * 128]

                def w3(k):
                    return wht[:, W3OFF + k * 3 : W3OFF + (k + 1) * 3]

                # prefetch next tile's inputs a full tile ahead
                if t + 1 < TILES:
                    inp = load_inputs(t + 1)

                # drain speed_in L1 (ACT) while the PE starts on this tile's
                # embedding matmuls
                hst = acts.tile([128, 2, NT], sdt, tag="h", name=f"hst{t}")
                for m in range(2):
                    nc.scalar.activation(
                        hst[:, m, :], ph[m][:], Relu, bias=sib1_a(m)
                    )

                # branch L1 over the embedding K-tiles (k=0..3); the 5th
                # K-tile (s) joins once computed.
                p1 = [psum.tile([128, NT], f32, tag="ps", name=f"p1_{t}_{i}") for i in range(MH)]
                p4 = [psum.tile([128, NT], f32, tag="ps", name=f"p4_{t}_{i}") for i in range(MH)]
                for m in range(MH):
                    for k in range(KE):
                        nc.tensor.matmul(
                            p1[m][:],
                            lhsT=w1(k, m),
                            rhs=xt[:, k, :],
                            start=(k == 0),
                            stop=False,
                        )

                # speed_in L2 (no relu): s_T[l,b]
                ps = psum.tile([128, NT], f32, tag="ps", name=f"ps{t}")
                for k in range(KH):
                    nc.tensor.matmul(
                        ps[:],
                        lhsT=siW2_a(k),
                        rhs=hst[:, k, :],
                        start=(k == 0),
                        stop=(k == KH - 1),
                    )
                sst = acts.tile([128, NT], sdt, tag="s", name=f"sst{t}")
                nc.scalar.activation(sst[:], ps[:], Ident, bias=sib2_a)

                # speed-head L1 embedding K-tiles keep the PE busy while ACT
                # produces s
                for m in range(MH):
                    for k in range(KE):
                        nc.tensor.matmul(
                            p4[m][:],
                            lhsT=soW1_a(k, m),
                            rhs=xt[:, k, :],
                            start=(k == 0),
                            stop=False,
                        )

                # close the L1 groups with the s K-tile; drain each group the
                # moment its last matmul lands
                a1 = acts.tile([128, 2, NT], sdt, tag="a1", name=f"a1_{t}")
                q1 = acts.tile([128, 2, NT], sdt, tag="q1", name=f"q1_{t}")
                for m in range(MH):
                    nc.tensor.matmul(
                        p1[m][:], lhsT=w1(KE, m), rhs=sst[:], start=False, stop=True
                    )
                for m in range(MH):
                    nc.vector.tensor_scalar(
                        a1[:, m, :], p1[m][:], bht[:, m : m + 1], 0.0, add, mx
                    )
                for m in range(MH):
                    nc.tensor.matmul(
                        p4[m][:],
                        lhsT=soW1_a(KE, m),
                        rhs=sst[:],
                        start=False,
                        stop=True,
                    )

                # next tile's speed_in L1 fills the PE while the DVE drains a1
                if t + 1 < TILES:
                    ph = emit_ph(t + 1, inp[1])

                for m in range(MH):
                    nc.vector.tensor_scalar(
                        q1[:, m, :], p4[m][:], sob1_a(m), 0.0, add, mx
                    )

                # L2 for both heads, drains interleaved so the following
                # matmul stream hides each drain
                a2 = acts.tile([128, 2, NT], sdt, tag="a2", name=f"a2_{t}")
                q2 = acts.tile([128, 2, NT], sdt, tag="q2", name=f"q2_{t}")
                p2 = [psum.tile([128, NT], f32, tag="ps", name=f"p2_{t}_{i}") for i in range(MH)]
                p5 = [psum.tile([128, NT], f32, tag="ps", name=f"p5_{t}_{i}") for i in range(MH)]
                for m in range(MH):
                    for k in range(KH):
                        nc.tensor.matmul(
                            p2[m][:],
                            lhsT=w2(k, m),
                            rhs=a1[:, k, :],
                            start=(k == 0),
                            stop=(k == KH - 1),
                        )
                    nc.vector.tensor_scalar(
                        a2[:, m, :], p2[m][:], bht[:, 2 + m : 3 + m], 0.0, add, mx
                    )
                for m in range(MH):
                    for k in range(KH):
                        nc.tensor.matmul(
                            p5[m][:],
                            lhsT=soW2_a(k, m),
                            rhs=q1[:, k, :],
                            start=(k == 0),
                            stop=(k == KH - 1),
                        )
                    # q2 drains on ACT so they run in parallel with the a2
                    # drains on DVE — keeps L3/SP3 fed without PE gaps
                    nc.scalar.activation(
                        q2[:, m, :], p5[m][:], Relu, bias=sob2_a(m)
                    )

                # L3: branch logits -> sigmoid; speed head -> identity
                p3 = psum.tile([3, NT], f32, tag="ps", name=f"p3_{t}")
                for k in range(KH):
                    nc.tensor.matmul(
                        p3[:],
                        lhsT=w3(k),
                        rhs=a2[:, k, :],
                        start=(k == 0),
                        stop=(k == KH - 1),
                    )
                cout = outs.tile([3, NT], f32, tag="c", name=f"cout{t}")
                nc.scalar.activation(cout[:], p3[:], Sig, bias=bht[0:3, 4:5])
                nc.sync.dma_start(out=ctrl[:, bsl], in_=cout[:])

                p6 = psum.tile([1, NT], f32, tag="ps", name=f"p6_{t}")
                for k in range(KH):
                    nc.tensor.matmul(
                        p6[:],
                        lhsT=soW3_a(k),
                        rhs=q2[:, k, :],
                        start=(k == 0),
                        stop=(k == KH - 1),
                    )
                sout = outs.tile([1, NT], f32, tag="sv", name=f"sout{t}")
                nc.scalar.activation(sout[:], p6[:], Ident, bias=sob3_s[0:1, 0:1])
                nc.sync.dma_start(out=spd[0:1, bsl], in_=sout[:])

    nc.finalize()
    return nc


def _get_nc():
    if "nc" not in _cached:
        _cached["nc"] = _build()
    return _cached["nc"]


def _route(command):
    """Bucket samples by command into single-command tiles of NT.

    Returns (slots, tile_cmd, nreal): slots[NCORES*CAP] maps device slot ->
    original sample index (bucket tails padded with a repeated in-bucket
    sample; trailing dummy tiles use sample 0), tile_cmd[NCORES*TILES] gives
    each tile's branch id, and slots[nreal:] are dummy-tile slots whose
    outputs must not be scattered back.
    """
    cmd = np.clip(np.asarray(command).astype(np.int64) - 1, 0, NB - 1)
    order = np.argsort(cmd, kind="stable")
    counts = np.bincount(cmd, minlength=NB)
    pieces = []
    tile_cmd = []
    pos = 0
    for c in range(NB):
        n = int(counts[c])
        if n == 0:
            continue
        idxs = order[pos : pos + n]
        pos += n
        ntile = -(-n // NT)
        pad = ntile * NT - n
        if pad:
            idxs = np.concatenate([idxs, np.full(pad, idxs[0], np.int64)])
        pieces.append(idxs)
        tile_cmd.extend([c] * ntile)
    nreal = len(tile_cmd) * NT
    ndum = NCORES * TILES - len(tile_cmd)
    assert ndum >= 0, "tile budget exceeded"
    if ndum:
        pieces.append(np.zeros(ndum * NT, np.int64))
        tile_cmd.extend([0] * ndum)
    return np.concatenate(pieces), np.asarray(tile_cmd), nreal


def _prep_in_maps(inputs, slots, tile_cmd):
    import ml_dtypes

    sdt_np = ml_dtypes.bfloat16 if COMPUTE_MODE == "bf16" else np.float32

    def s(x):  # storage-dtype cast
        return np.ascontiguousarray(np.asarray(x, np.float32).astype(sdt_np))

    def f(x):  # always-f32 (biases)
        return np.ascontiguousarray(np.asarray(x, np.float32))

    emb = np.asarray(inputs["embedding"], np.float32)
    speed = np.asarray(inputs["speed"], np.float32)

    gx = emb[slots]                                  # [NCORES*CAP, E]
    gs = speed[slots, 0]                             # [NCORES*CAP]

    # packed per-command weight/bias images (gathered per tile below)
    bW1 = np.asarray(inputs["bW1"], np.float32)
    bb1 = np.asarray(inputs["bb1"], np.float32)
    bW2 = np.asarray(inputs["bW2"], np.float32)
    bb2 = np.asarray(inputs["bb2"], np.float32)
    bW3 = np.asarray(inputs["bW3"], np.float32)
    bb3 = np.asarray(inputs["bb3"], np.float32)
    pw = np.empty((NB, 128, WCOLS), np.float32)
    pb = np.zeros((NB, 128, 5), np.float32)
    for c in range(NB):
        w1 = bW1[c].reshape(KD, 128, H).transpose(1, 0, 2).reshape(128, KD * H)
        w2 = bW2[c].reshape(KH, 128, H).transpose(1, 0, 2).reshape(128, KH * H)
        w3 = bW3[c].reshape(KH, 128, 3).transpose(1, 0, 2).reshape(128, KH * 3)
        pw[c] = np.concatenate([w1, w2, w3], axis=1)
        pb[c, :, 0:2] = bb1[c].reshape(2, 128).T
        pb[c, :, 2:4] = bb2[c].reshape(2, 128).T
        pb[c, 0:3, 4] = bb3[c]
    pw = pw.astype(sdt_np)

    # packed shared-weight image [128, FWCOLS] and bias image [128, 7]
    soW1p = (
        np.asarray(inputs["so_W1"], np.float32)
        .reshape(KD, 128, H).transpose(1, 0, 2).reshape(128, KD * H)
    )
    soW2p = (
        np.asarray(inputs["so_W2"], np.float32)
        .reshape(KH, 128, H).transpose(1, 0, 2).reshape(128, KH * H)
    )
    siW2p = (
        np.asarray(inputs["si_W2"], np.float32)
        .reshape(KH, 128, SL).transpose(1, 0, 2).reshape(128, KH * SL)
    )
    soW3p = (
        np.asarray(inputs["so_W3"], np.float32)
        .reshape(KH, 128, 1).transpose(1, 0, 2).reshape(128, KH)
    )
    fwp = np.concatenate([soW1p, soW2p, siW2p, soW3p], axis=1)
    fbp = np.zeros((128, 7), np.float32)
    fbp[:, 0:2] = np.asarray(inputs["si_b1"], np.float32).reshape(2, 128).T
    fbp[:, 2:4] = np.asarray(inputs["so_b1"], np.float32).reshape(2, 128).T
    fbp[:, 4:6] = np.asarray(inputs["so_b2"], np.float32).reshape(2, 128).T
    fbp[:, 6] = np.asarray(inputs["si_b2"], np.float32).reshape(1, 128)[0]
    shared = {
        "fw": s(fwp),
        "fb": f(fbp),
        "siW1": s(inputs["si_W1"].reshape(1, H)),
        "sob3": f(inputs["so_b3"].reshape(1, 1)),
    }

    in_maps = []
    for c in range(NCORES):
        sl = slice(c * CAP, (c + 1) * CAP)
        tc_ = tile_cmd[c * TILES : (c + 1) * TILES]
        # [tile, partition(=dim within K-tile), K-tile, sample]
        xc = gx[sl].reshape(TILES, NT, KE, 128).transpose(0, 3, 2, 1)
        in_map = {
            "xh": s(xc),
            "sp": s(gs[sl].reshape(1, CAP)),
            "wh": np.ascontiguousarray(pw[tc_]),
            "bh": np.ascontiguousarray(pb[tc_]),
        }
        in_map.update(shared)
        in_maps.append(in_map)
    return in_maps


def _run(inputs, trace=False):
    from concourse.bass_utils import run_bass_kernel_spmd

    command = np.asarray(inputs["command"])
    assert command.shape == (B,), command.shape
    slots, tile_cmd, nreal = _route(command)
    in_maps = _prep_in_maps(inputs, slots, tile_cmd)

    nc = _get_nc()
    res = run_bass_kernel_spmd(nc, in_maps, list(range(NCORES)), trace=trace)

    control = np.zeros((B, 3), np.float32)
    speed_pred = np.zeros((B, 1), np.float32)
    for c in range(NCORES):
        lo = c * CAP
        nvalid = min(max(nreal - lo, 0), CAP)
        if nvalid == 0:
            continue
        sl = slots[lo : lo + nvalid]
        control[sl] = res.results[c]["ctrl"][:, :nvalid].T
        speed_pred[sl, 0] = res.results[c]["spd"][0, :nvalid]

    # Out-of-range commands select no branch in the reference -> zeros.
    bad = (command < 1) | (command > NB)
    if bad.any():
        control[bad] = 0.0
    return control, speed_pred, res


def kernel(**inputs):
    control, speed_pred, _ = _run(inputs, trace=False)
    return control, speed_pred


# revision 18
# speedup vs baseline: 1.4629x; 1.0605x over previous
"""Trainium2 Bass kernel for nn_CILRSVAEModel (CILRS-style MoE branch routing).

Model (per sample):
  s      = relu(speed @ si_W1 + si_b1) @ si_W2 + si_b2            # [SL]
  emb    = concat(embedding, s)                                   # [D=640]
  branch = command-selected MLP: sigmoid(relu(relu(emb@W1+b1)@W2+b2)@W3+b3)
  speed_pred = relu(relu(emb@so_W1+so_b1)@so_W2+so_b2)@so_W3+so_b3

Strategy: pure data parallel over batch, plus host-side MoE routing — samples
are bucketed by `command` into single-command tiles of 512, so the device
computes only the selected branch per sample (6x fewer branch FLOPs than the
dense reference). Each tile's branch weights are gathered host-side into a
per-tile packed weight image, keeping the device program fully static. All
device-facing arrays are packed host-side into [128-partition, contiguous]
layouts so every DMA is 128 descriptors of multi-KB contiguous runs, and the
embedding is pre-transposed so activations live in [feature, batch] layout
end-to-end (no on-device transposes). Matmuls run as lhsT=[K,M] weights x
rhs=[K,N] activations with N=512 batch columns, in fp32r (full PE rate).
PSUM->SBUF bias+activation drains are split between the Scalar (ACT) and
Vector (DVE) engines; relu/sigmoid/identity share one ACT table set so there
are no table reloads.
"""

import os

# The device program runs through jax's axon/PJRT backend; make sure a
# harness-set JAX_PLATFORMS=cpu doesn't hide the neuron devices.
if os.environ.get("JAX_PLATFORMS", None) in ("cpu",):
    os.environ.pop("JAX_PLATFORMS")

import sys

import numpy as np

for _p in ("/opt/trn_rl_repo",):
    if _p not in sys.path and os.path.isdir(_p):
        sys.path.insert(0, _p)

B = 65536
E = 512
SL = 128
H = 256
NB = 6
D = E + SL
NCORES = 8
NT = 512                    # samples per tile (= fp32 matmul moving-dim max)
TILES = 17                  # tiles per core; 8*17=136 >= worst case 134
CAP = NT * TILES            # samples per core incl. padding

KD = D // 128               # 5 K-tiles of the branch input dim
KE = E // 128               # 4 of them come straight from the embedding
KH = H // 128               # 2 K-tiles of the hidden dim
MH = H // 128               # 2 M-tiles of the hidden dim

# packed per-tile weight image: per partition p (=row within a K-tile):
#   [ W1 (KD*H) | W2 (KH*H) | W3 (KH*3) ]
W2OFF = KD * H              # 1280
W3OFF = W2OFF + KH * H      # 1792
WCOLS = W3OFF + KH * 3      # 1798

# packed shared-weight image column offsets
SOW2OFF = KD * H            # 1280
SIW2OFF = SOW2OFF + KH * H  # 1792
SOW3OFF = SIW2OFF + KH * SL  # 2048
FWCOLS = SOW3OFF + KH       # 2050

# "f32r": fp32 storage, fp32r (tf32-like full-rate) matmuls.
# "bf16": bf16 storage/matmuls (half the DMA), fp32 accumulate.
COMPUTE_MODE = os.environ.get("KERNEL_DT", "f32r")

_cached = {}


def _build():
    """Build + finalize the SPMD Bacc program (identical on all 8 cores)."""
    from concourse import bacc, mybir
    import concourse.tile as tile
    from concourse.bass import ts

    f32 = mybir.dt.float32
    bf16 = mybir.dt.bfloat16
    # storage dtype for everything the matmul touches. fp32r runs the PE at
    # full rate (vs 1/4 for fp32) with tf32-like precision; walrus requires
    # the whole producer chain (DMA, DVE) to carry the fp32r dtype, so the
    # tensors are declared fp32r end-to-end (numpy side is plain float32).
    sdt = bf16 if COMPUTE_MODE == "bf16" else mybir.dt.float32r

    add = mybir.AluOpType.add
    mx = mybir.AluOpType.max
    Relu = mybir.ActivationFunctionType.Relu
    Ident = mybir.ActivationFunctionType.Identity
    Sig = mybir.ActivationFunctionType.Sigmoid

    nc = bacc.Bacc(None, target_bir_lowering=False)

    # --- I/O ---------------------------------------------------------------
    xh = nc.declare_dram_parameter("xh", [TILES, 128, KE, NT], sdt, isOutput=False)
    sp = nc.declare_dram_parameter("sp", [1, CAP], sdt, isOutput=False)
    wh = nc.declare_dram_parameter("wh", [TILES, 128, WCOLS], sdt, isOutput=False)
    bh = nc.declare_dram_parameter("bh", [TILES, 128, 5], f32, isOutput=False)
    # packed shared weights: [ soW1 (KD*H) | soW2 (KH*H) | siW2 (KH*SL) | soW3 (KH) ]
    fw = nc.declare_dram_parameter("fw", [128, FWCOLS], sdt, isOutput=False)
    # packed shared biases: [ sib1 (2) | sob1 (2) | sob2 (2) | sib2 (1) ]
    fb = nc.declare_dram_parameter("fb", [128, 7], f32, isOutput=False)
    siW1 = nc.declare_dram_parameter("siW1", [1, H], sdt, isOutput=False)
    sob3 = nc.declare_dram_parameter("sob3", [1, 1], f32, isOutput=False)
    ctrl = nc.declare_dram_parameter("ctrl", [3, CAP], f32, isOutput=True)
    spd = nc.declare_dram_parameter("spd", [1, CAP], f32, isOutput=True)

    with tile.TileContext(nc) as tc:
        with (
            tc.tile_pool(name="fixed", bufs=1) as fixed,
            tc.tile_pool(name="wts", bufs=3) as wts,
            tc.tile_pool(name="xin", bufs=3) as xin,
            tc.tile_pool(name="acts", bufs=2) as acts,
            tc.tile_pool(name="outs", bufs=3) as outs,
            tc.tile_pool(name="psum", bufs=8, space="PSUM") as psum,
        ):
            def load_inputs(t):
                """Issue the input DMAs for tile t (one tile of prefetch)."""
                xt = xin.tile([128, KE, NT], sdt, tag="xt", name=f"xt{t}")
                nc.sync.dma_start(out=xt[:], in_=xh[t])
                spt = xin.tile([1, NT], sdt, tag="spt", name=f"spt{t}")
                nc.sync.dma_start(out=spt[:], in_=sp[0:1, ts(t, NT)])
                wht = wts.tile([128, WCOLS], sdt, tag="w", name=f"wht{t}")
                nc.sync.dma_start(out=wht[:], in_=wh[t])
                bht = wts.tile([128, 5], f32, tag="b", name=f"bht{t}")
                nc.sync.dma_start(out=bht[:], in_=bh[t])
                return xt, spt, wht, bht

            def emit_ph(t, spt):
                """speed_in L1 matmuls for tile t (K=1 outer products)."""
                ph = [
                    psum.tile([128, NT], f32, tag="ps", name=f"ph{t}_{i}")
                    for i in range(2)
                ]
                for m in range(2):
                    nc.tensor.matmul(
                        ph[m][:],
                        lhsT=siW1_s[0:1, ts(m, 128)],
                        rhs=spt[:],
                        start=True,
                        stop=True,
                    )
                return ph

            # tile-0 inputs first so compute starts as soon as they land;
            # the packed shared-weight images load in parallel on other
            # queues and are first needed a few matmul-groups in.
            inp = load_inputs(0)
            siW1_s = fixed.tile([1, H], sdt)
            nc.sync.dma_start(out=siW1_s[:], in_=siW1[:])
            fb_s = fixed.tile([128, 7], f32)
            nc.sync.dma_start(out=fb_s[:], in_=fb[:])
            sob3_s = fixed.tile([1, 1], f32)
            nc.sync.dma_start(out=sob3_s[:], in_=sob3[:])
            # the big shared-weight image is first needed a few matmul groups
            # into tile 0 — keep it off the critical xt0/wht0 transfer path
            fw_s = fixed.tile([128, FWCOLS], sdt)

            def soW1_a(k, m):
                return fw_s[:, k * H + m * 128 : k * H + (m + 1) * 128]

            def soW2_a(k, m):
                return fw_s[:, SOW2OFF + k * H + m * 128 : SOW2OFF + k * H + (m + 1) * 128]

            def siW2_a(k):
                return fw_s[:, SIW2OFF + k * SL : SIW2OFF + (k + 1) * SL]

            def soW3_a(k):
                return fw_s[:, SOW3OFF + k : SOW3OFF + k + 1]

            def sib1_a(m):
                return fb_s[:, m : m + 1]

            def sob1_a(m):
                return fb_s[:, 2 + m : 3 + m]

            def sob2_a(m):
                return fb_s[:, 4 + m : 5 + m]

            sib2_a = fb_s[:, 6:7]

            ph = emit_ph(0, inp[1])
            nc.sync.dma_start(out=fw_s[:], in_=fw[:])

            for t in range(TILES):
                bsl = ts(t, NT)
                xt, spt, wht, bht = inp

                def w1(k, m):
                    return wht[:, k * H + m * 128 : k * H + (m + 1) * 128]

                def w2(k, m):
                    return wht[:, W2OFF + k * H + m * 128 : W2OFF + k * H + (m + 1) * 128]

                def w3(k):
                    return wht[:, W3OFF + k * 3 : W3OFF + (k + 1) * 3]

                # prefetch next tile's inputs a full tile ahead
                if t + 1 < TILES:
                    inp = load_inputs(t + 1)

                # drain speed_in L1 (ACT) while the PE starts on this tile's
                # embedding matmuls
                hst = acts.tile([128, 2, NT], sdt, tag="h", name=f"hst{t}")
                nc.vector.tensor_scalar(
                    hst[:, 0, :], ph[0][:], sib1_a(0), 0.0, add, mx
                )
                nc.scalar.activation(hst[:, 1, :], ph[1][:], Relu, bias=sib1_a(1))

                # branch L1 over the embedding K-tiles (k=0..3); the 5th
                # K-tile (s) joins once computed.
                p1 = [psum.tile([128, NT], f32, tag="ps", name=f"p1_{t}_{i}") for i in range(MH)]
                p4 = [psum.tile([128, NT], f32, tag="ps", name=f"p4_{t}_{i}") for i in range(MH)]
                for m in range(MH):
                    for k in range(KE):
                        nc.tensor.matmul(
                            p1[m][:],
                            lhsT=w1(k, m),
                            rhs=xt[:, k, :],
                            start=(k == 0),
                            stop=False,
                        )

                # speed_in L2 (no relu): s_T[l,b]
                ps = psum.tile([128, NT], f32, tag="ps", name=f"ps{t}")
                for k in range(KH):
                    nc.tensor.matmul(
                        ps[:],
                        lhsT=siW2_a(k),
                        rhs=hst[:, k, :],
                        start=(k == 0),
                        stop=(k == KH - 1),
                    )
                sst = acts.tile([128, NT], sdt, tag="s", name=f"sst{t}")
                nc.scalar.activation(sst[:], ps[:], Ident, bias=sib2_a)

                # speed-head L1 embedding K-tiles keep the PE busy while ACT
                # produces s
                for m in range(MH):
                    for k in range(KE):
                        nc.tensor.matmul(
                            p4[m][:],
                            lhsT=soW1_a(k, m),
                            rhs=xt[:, k, :],
                            start=(k == 0),
                            stop=False,
                        )

                # close the L1 groups with the s K-tile; drain each group the
                # moment its last matmul lands
                a1 = acts.tile([128, 2, NT], sdt, tag="a1", name=f"a1_{t}")
                q1 = acts.tile([128, 2, NT], sdt, tag="q1", name=f"q1_{t}")
                for m in range(MH):
                    nc.tensor.matmul(
                        p1[m][:], lhsT=w1(KE, m), rhs=sst[:], start=False, stop=True
                    )
                nc.vector.tensor_scalar(
                    a1[:, 0, :], p1[0][:], bht[:, 0:1], 0.0, add, mx
                )
                nc.scalar.activation(a1[:, 1, :], p1[1][:], Relu, bias=bht[:, 1:2])
                for m in range(MH):
                    nc.tensor.matmul(
                        p4[m][:],
                        lhsT=soW1_a(KE, m),
                        rhs=sst[:],
                        start=False,
                        stop=True,
                    )

                # next tile's speed_in L1 fills the PE while the DVE drains a1
                if t + 1 < TILES:
                    ph = emit_ph(t + 1, inp[1])

                nc.vector.tensor_scalar(
                    q1[:, 0, :], p4[0][:], sob1_a(0), 0.0, add, mx
                )
                nc.scalar.activation(q1[:, 1, :], p4[1][:], Relu, bias=sob1_a(1))

                # L2 for both heads, drains interleaved so the following
                # matmul stream hides each drain
                a2 = acts.tile([128, 2, NT], sdt, tag="a2", name=f"a2_{t}")
                q2 = acts.tile([128, 2, NT], sdt, tag="q2", name=f"q2_{t}")
                p2 = [psum.tile([128, NT], f32, tag="ps", name=f"p2_{t}_{i}") for i in range(MH)]
                p5 = [psum.tile([128, NT], f32, tag="ps", name=f"p5_{t}_{i}") for i in range(MH)]
                for m in range(MH):
                    for k in range(KH):
                        nc.tensor.matmul(
                            p2[m][:],
                            lhsT=w2(k, m),
                            rhs=a1[:, k, :],
                            start=(k == 0),
                            stop=(k == KH - 1),
                        )
                    if m == 0:
                        nc.vector.tensor_scalar(
                            a2[:, 0, :], p2[0][:], bht[:, 2:3], 0.0, add, mx
                        )
                    else:
                        nc.scalar.activation(
                            a2[:, 1, :], p2[1][:], Relu, bias=bht[:, 3:4]
                        )
                for m in range(MH):
                    for k in range(KH):
                        nc.tensor.matmul(
                            p5[m][:],
                            lhsT=soW2_a(k, m),
                            rhs=q1[:, k, :],
                            start=(k == 0),
                            stop=(k == KH - 1),
                        )
                    # drains paired across DVE/ACT so both halves finish in
                    # parallel and L3/SP3 start without PE gaps
                    if m == 0:
                        nc.vector.tensor_scalar(
                            q2[:, 0, :], p5[0][:], sob2_a(0), 0.0, add, mx
                        )
                    else:
                        nc.scalar.activation(
                            q2[:, 1, :], p5[1][:], Relu, bias=sob2_a(1)
                        )

                # L3: branch logits -> sigmoid; speed head -> identity
                p3 = psum.tile([3, NT], f32, tag="ps", name=f"p3_{t}")
                for k in range(KH):
                    nc.tensor.matmul(
                        p3[:],
                        lhsT=w3(k),
                        rhs=a2[:, k, :],
                        start=(k == 0),
                        stop=(k == KH - 1),
                    )
                cout = outs.tile([3, NT], f32, tag="c", name=f"cout{t}")
                nc.scalar.activation(cout[:], p3[:], Sig, bias=bht[0:3, 4:5])
                nc.sync.dma_start(out=ctrl[:, bsl], in_=cout[:])

                p6 = psum.tile([1, NT], f32, tag="ps", name=f"p6_{t}")
                for k in range(KH):
                    nc.tensor.matmul(
                        p6[:],
                        lhsT=soW3_a(k),
                        rhs=q2[:, k, :],
                        start=(k == 0),
                        stop=(k == KH - 1),
                    )
                sout = outs.tile([1, NT], f32, tag="sv", name=f"sout{t}")
                nc.scalar.activation(sout[:], p6[:], Ident, bias=sob3_s[0:1, 0:1])
                nc.sync.dma_start(out=spd[0:1, bsl], in_=sout[:])

    nc.finalize()
    return nc


def _get_nc():
    if "nc" not in _cached:
        _cached["nc"] = _build()
    return _cached["nc"]


def _route(command):
    """Bucket samples by command into single-command tiles of NT.

    Returns (slots, tile_cmd, nreal): slots[NCORES*CAP] maps device slot ->
    original sample index (bucket tails padded with a repeated in-bucket
    sample; trailing dummy tiles use sample 0), tile_cmd[NCORES*TILES] gives
    each tile's branch id, and slots[nreal:] are dummy-tile slots whose
    outputs must not be scattered back.
    """
    cmd = np.clip(np.asarray(command).astype(np.int64) - 1, 0, NB - 1)
    order = np.argsort(cmd, kind="stable")
    counts = np.bincount(cmd, minlength=NB)
    pieces = []
    tile_cmd = []
    pos = 0
    for c in range(NB):
        n = int(counts[c])
        if n == 0:
            continue
        idxs = order[pos : pos + n]
        pos += n
        ntile = -(-n // NT)
        pad = ntile * NT - n
        if pad:
            idxs = np.concatenate([idxs, np.full(pad, idxs[0], np.int64)])
        pieces.append(idxs)
        tile_cmd.extend([c] * ntile)
    nreal = len(tile_cmd) * NT
    ndum = NCORES * TILES - len(tile_cmd)
    assert ndum >= 0, "tile budget exceeded"
    if ndum:
        pieces.append(np.zeros(ndum * NT, np.int64))
        tile_cmd.extend([0] * ndum)
    return np.concatenate(pieces), np.asarray(tile_cmd), nreal


def _prep_in_maps(inputs, slots, tile_cmd):
    import ml_dtypes

    sdt_np = ml_dtypes.bfloat16 if COMPUTE_MODE == "bf16" else np.float32

    def s(x):  # storage-dtype cast
        return np.ascontiguousarray(np.asarray(x, np.float32).astype(sdt_np))

    def f(x):  # always-f32 (biases)
        return np.ascontiguousarray(np.asarray(x, np.float32))

    emb = np.asarray(inputs["embedding"], np.float32)
    speed = np.asarray(inputs["speed"], np.float32)

    gx = emb[slots]                                  # [NCORES*CAP, E]
    gs = speed[slots, 0]                             # [NCORES*CAP]

    # packed per-command weight/bias images (gathered per tile below)
    bW1 = np.asarray(inputs["bW1"], np.float32)
    bb1 = np.asarray(inputs["bb1"], np.float32)
    bW2 = np.asarray(inputs["bW2"], np.float32)
    bb2 = np.asarray(inputs["bb2"], np.float32)
    bW3 = np.asarray(inputs["bW3"], np.float32)
    bb3 = np.asarray(inputs["bb3"], np.float32)
    pw = np.empty((NB, 128, WCOLS), np.float32)
    pb = np.zeros((NB, 128, 5), np.float32)
    for c in range(NB):
        w1 = bW1[c].reshape(KD, 128, H).transpose(1, 0, 2).reshape(128, KD * H)
        w2 = bW2[c].reshape(KH, 128, H).transpose(1, 0, 2).reshape(128, KH * H)
        w3 = bW3[c].reshape(KH, 128, 3).transpose(1, 0, 2).reshape(128, KH * 3)
        pw[c] = np.concatenate([w1, w2, w3], axis=1)
        pb[c, :, 0:2] = bb1[c].reshape(2, 128).T
        pb[c, :, 2:4] = bb2[c].reshape(2, 128).T
        pb[c, 0:3, 4] = bb3[c]
    pw = pw.astype(sdt_np)

    # packed shared-weight image [128, FWCOLS] and bias image [128, 7]
    soW1p = (
        np.asarray(inputs["so_W1"], np.float32)
        .reshape(KD, 128, H).transpose(1, 0, 2).reshape(128, KD * H)
    )
    soW2p = (
        np.asarray(inputs["so_W2"], np.float32)
        .reshape(KH, 128, H).transpose(1, 0, 2).reshape(128, KH * H)
    )
    siW2p = (
        np.asarray(inputs["si_W2"], np.float32)
        .reshape(KH, 128, SL).transpose(1, 0, 2).reshape(128, KH * SL)
    )
    soW3p = (
        np.asarray(inputs["so_W3"], np.float32)
        .reshape(KH, 128, 1).transpose(1, 0, 2).reshape(128, KH)
    )
    fwp = np.concatenate([soW1p, soW2p, siW2p, soW3p], axis=1)
    fbp = np.zeros((128, 7), np.float32)
    fbp[:, 0:2] = np.asarray(inputs["si_b1"], np.float32).reshape(2, 128).T
    fbp[:, 2:4] = np.asarray(inputs["so_b1"], np.float32).reshape(2, 128).T
    fbp[:, 4:6] = np.asarray(inputs["so_b2"], np.float32).reshape(2, 128).T
    fbp[:, 6] = np.asarray(inputs["si_b2"], np.float32).reshape(1, 128)[0]
    shared = {
        "fw": s(fwp),
        "fb": f(fbp),
        "siW1": s(inputs["si_W1"].reshape(1, H)),
        "sob3": f(inputs["so_b3"].reshape(1, 1)),
    }

    in_maps = []
    for c in range(NCORES):
        sl = slice(c * CAP, (c + 1) * CAP)
        tc_ = tile_cmd[c * TILES : (c + 1) * TILES]
        # [tile, partition(=dim within K-tile), K-tile, sample]
        xc = gx[sl].reshape(TILES, NT, KE, 128).transpose(0, 3, 2, 1)
        in_map = {
            "xh": s(xc),
            "sp": s(gs[sl].reshape(1, CAP)),
            "wh": np.ascontiguousarray(pw[tc_]),
            "bh": np.ascontiguousarray(pb[tc_]),
        }
        in_map.update(shared)
        in_maps.append(in_map)
    return in_maps


def _run(inputs, trace=False):
    from concourse.bass_utils import run_bass_kernel_spmd

    command = np.asarray(inputs["command"])
    assert command.shape == (B,), command.shape
    slots, tile_cmd, nreal = _route(command)
    in_maps = _prep_in_maps(inputs, slots, tile_cmd)

    nc = _get_nc()
    res = run_bass_kernel_spmd(nc, in_maps, list(range(NCORES)), trace=trace)

    control = np.zeros((B, 3), np.float32)
    speed_pred = np.zeros((B, 1), np.float32)
    for c in range(NCORES):
        lo = c * CAP
        nvalid = min(max(nreal - lo, 0), CAP)
        if nvalid == 0:
            continue
        sl = slots[lo : lo + nvalid]
        control[sl] = res.results[c]["ctrl"][:, :nvalid].T
        speed_pred[sl, 0] = res.results[c]["spd"][0, :nvalid]

    # Out-of-range commands select no branch in the reference -> zeros.
    bad = (command < 1) | (command > NB)
    if bad.any():
        control[bad] = 0.0
    return control, speed_pred, res


def kernel(**inputs):
    control, speed_pred, _ = _run(inputs, trace=False)
    return control, speed_pred
